# revision 1
# baseline (speedup 1.0000x reference)
"""Affinity-propagation (CSPN-3D) Trainium2 kernel.

Problem: guidance [24,256,256,32] f32, blur [1,256,256,32] f32.
3 iterations of (x-plane, y-plane, z-plane) 8-neighbor gated propagation:

out(q) = r(q) + c1(q) * [ sum_k G_k(q+d_k) * r(q+d_k) - S(q) * r(q) ]
  A(q) = sum_k |G_k(q+d_k)|,  S(q) = sum_k G_k(q+d_k),  c1 = 1/max(A,eps)
(equivalent to the reference's  (1-S/A)*r + (1/A)*sum_k G_k(q+d)*r(q+d))

Sharding: 8 cores, X sharded 32 rows/core with ghost margin 5,
communication free. Step 1 (the 6th X-crossing step) uses host-staggered
blur tiles + an unbaked gate-stream variant so it consumes no margin.

Layout (per core): partitions p = yb*42 + xl (3 y-thirds x 42 x-rows = 126),
free = (yc 88 = 86+2 overlap, zc 34 = 32+2 zero pad) -> FD 2992.
Gate fields are host-pre-shifted by their full neighbor offset d_k (plus the
inverse +-1 partition shift), so all device math is partition-aligned
elementwise; a PE shift-matmul accumulates the 9 slot products (8 neighbor
terms + the -S*r term) into PSUM in f32, routing the +-1 x-shift groups back
into place.
"""

import numpy as np
import ml_dtypes

BF = ml_dtypes.bfloat16

X = Y = 256
Z = 32
NCORES = 8
W = X // NCORES          # 32 interior rows per core
M = 5                    # ghost margin
S = W + 2 * M            # 42 slab rows
NYB = 3                  # y thirds
YT = 86                  # y third width
YC = YT + 2              # y cols incl 2 overlap
ZC = Z + 2               # z cols incl 2 pads
FD = YC * ZC             # 2992
P = NYB * S              # 126 partitions
NCHUNK = 4
CF = FD // NCHUNK        # 748
NHALF = 2
HF = FD // NHALF         # 1496
GUARD = 36
SLOTF = GUARD + FD + GUARD  # 3064, even
PROP_TIME = 3

# k -> (dH, dW) neighbor offsets, matching reference PADS
DLIST = [(1, 1), (1, 0), (1, -1), (0, 1), (0, -1), (-1, 1), (-1, 0), (-1, -1)]
# slot order: groups by da in {-1,0,+1}, db in {-1,0,+1} (center group 2 slots)
SLOT_DADB = [(-1, -1), (-1, 0), (-1, 1), (0, -1), (0, 1), (1, -1), (1, 0), (1, 1)]
GROUP_SLOTS = [(0, 3), (3, 5), (5, 8)]  # slot ranges per group (da=-1,0,+1)

AXES = ["x", "y", "z"]


def _axis_slots(axis):
    """Return list of 8 (channel, dx, dy, dz) in slot order for this axis."""
    base = {"x": 0, "y": 8, "z": 16}[axis]
    out = []
    for (da, db) in SLOT_DADB:
        dH, dW = da, db
        k = DLIST.index((dH, dW))
        if axis == "x":
            d = (dH, dW, 0)
        elif axis == "y":
            d = (dH, 0, dW)
        else:
            d = (0, dH, dW)
        out.append((base + k,) + d)
    return out


def _shift_full(f, dx, dy, dz):
    """Zero-padded shift: out[x,y,z] = f[x+dx, y+dy, z+dz]."""
    o = np.zeros_like(f)
    tx0, tx1 = max(0, -dx), min(X, X - dx)
    ty0, ty1 = max(0, -dy), min(Y, Y - dy)
    tz0, tz1 = max(0, -dz), min(Z, Z - dz)
    o[tx0:tx1, ty0:ty1, tz0:tz1] = f[tx0 + dx:tx1 + dx, ty0 + dy:ty1 + dy,
                                     tz0 + dz:tz1 + dz]
    return o


def _slab_L(f, x0):
    """Full field [X,Y,Z] -> core slab in L layout [P, YC, ZC] (f32)."""
    pf = np.zeros((S, Y + 4, ZC), dtype=np.float32)
    r0, r1 = x0 - M, x0 - M + S
    c0, c1 = max(0, r0), min(X, r1)
    pf[c0 - r0:c1 - r0, 1:Y + 1, 1:Z + 1] = f[c0:c1]
    return np.concatenate([pf[:, i * YT:i * YT + YC, :] for i in range(NYB)], axis=0)


_COMPILED = None
_LAST_RESULTS = None


def _build_program():
    import concourse.bacc as bacc
    import concourse.mybir as mybir
    import concourse.tile as tile

    f32 = mybir.dt.float32
    bf16 = mybir.dt.bfloat16
    MULT = mybir.AluOpType.mult
    ADD = mybir.AluOpType.add

    nc = bacc.Bacc("TRN2", target_bir_lowering=False, debug=False,
                   num_devices=NCORES)

    # ---- DRAM I/O ----
    gs = {a: nc.dram_tensor(f"gs_{a}", [NCHUNK, P, 8, CF], bf16,
                            kind="ExternalInput").ap() for a in AXES}
    gu = {a: nc.dram_tensor(f"gu_{a}", [NCHUNK, P, 8, CF], bf16,
                            kind="ExternalInput").ap() for a in AXES}
    r0_in = nc.dram_tensor("r0", [P, YC, ZC], f32, kind="ExternalInput").ap()
    r0stag = nc.dram_tensor("r0stag", [NCHUNK, P, 3, 3, CF], bf16,
                            kind="ExternalInput").ap()
    shm = nc.dram_tensor("shm", [128, 3, 128], bf16, kind="ExternalInput").ap()
    rout = nc.dram_tensor("rout", [P, YC, ZC], f32, kind="ExternalOutput").ap()

    with tile.TileContext(nc) as tc:
        with tc.tile_pool(name="stat", bufs=1) as st, \
             tc.tile_pool(name="work", bufs=1) as wk, \
             tc.tile_pool(name="fin", bufs=2) as fin, \
             tc.tile_pool(name="io", bufs=2) as io, \
             tc.tile_pool(name="psum", bufs=2, space="PSUM") as pp:

            # ---- static tiles ----
            t_r = st.tile([P, YC, ZC], f32, tag="r", name="t_r")
            t_r3 = st.tile([P, 3, SLOTF], bf16, tag="r3", name="t_r3")
            t_c1b = {a: st.tile([P, FD], bf16, tag=f"c1b{a}", name=f"t_c1b{a}")
                     for a in AXES}
            t_c0 = {a: st.tile([P, FD], f32, tag=f"c0{a}", name=f"t_c0{a}")
                    for a in AXES}
            t_shm = st.tile([128, 3, 128], bf16, tag="shm", name="t_shm")
            t_g = [st.tile([P, 8, CF], bf16, tag=f"gbuf{i}", name=f"t_g{i}")
                   for i in range(2)]
            t_carry = st.tile([P, FD], f32, tag="carry", name="t_carry")
            t_p = [st.tile([P, 8, CF], bf16, tag=f"pbuf{i}", name=f"t_p{i}")
                   for i in range(2)]

            nc.sync.dma_start(out=t_shm[:], in_=shm[:])
            nc.sync.dma_start(out=t_r[:], in_=r0_in[:])
            nc.gpsimd.memset(t_r3[:], 0.0)

            # ---- gate prep: per axis, per half, per CF2 sub-slice ----
            # A = sum|G(+d)|, S = sum G(+d), c1 = 1/max(A,eps), nS = -S
            CF2 = CF // 2

            def prep_axis(a):
                for ci in range(NCHUNK):
                    tgio = io.tile([P, 8, CF], bf16, tag="prepg", name="tgio")
                    dmae = nc.sync if ci % 2 == 0 else nc.scalar
                    dmae.dma_start(out=tgio[:], in_=gu[a][ci])
                    for h in range(CF // CF2):
                        hsl = slice(h * CF2, (h + 1) * CF2)
                        csl = slice(ci * CF + h * CF2, ci * CF + (h + 1) * CF2)
                        tg = tgio[:, :, hsl]
                        tabs = wk.tile([P, 8, CF2], bf16, tag="prepabs",
                                       name="tabs")
                        nc.vector.tensor_scalar(
                            tabs[:].bitcast(mybir.dt.int16),
                            tg.bitcast(mybir.dt.int16), 0x7FFF, None,
                            mybir.AluOpType.bitwise_and)
                        # A tree: L1 bf16, then f32
                        pa = wk.tile([P, 4, CF2], bf16, tag="prep_pa", name="pa")
                        nc.vector.tensor_tensor(out=pa[:], in0=tabs[:, 0:8:2, :],
                                                in1=tabs[:, 1:8:2, :], op=ADD)
                        pa2 = wk.tile([P, 2, CF2], f32, tag="prep_pa2",
                                      name="pa2")
                        nc.vector.tensor_tensor(out=pa2[:], in0=pa[:, 0:4:2, :],
                                                in1=pa[:, 1:4:2, :], op=ADD)
                        tA = wk.tile([P, CF2], f32, tag="prep_A", name="tA")
                        nc.vector.tensor_tensor(out=tA[:], in0=pa2[:, 0, :],
                                                in1=pa2[:, 1, :], op=ADD)
                        # S tree: L1 on gpsimd, rest gpsimd; nS = -S in bf16
                        ps1 = wk.tile([P, 4, CF2], bf16, tag="prep_ps",
                                      name="ps1")
                        nc.gpsimd.tensor_tensor(out=ps1[:], in0=tg[:, 0:8:2, :],
                                                in1=tg[:, 1:8:2, :], op=ADD)
                        ps2 = wk.tile([P, 2, CF2], f32, tag="prep_ps2",
                                      name="ps2")
                        nc.vector.tensor_tensor(out=ps2[:], in0=ps1[:, 0:4:2, :],
                                                in1=ps1[:, 1:4:2, :], op=ADD)
                        tS = wk.tile([P, CF2], f32, tag="prep_S", name="tS")
                        nc.gpsimd.tensor_tensor(out=tS[:], in0=ps2[:, 0, :],
                                                in1=ps2[:, 1, :], op=ADD)
                        # c1 = 1/max(A, eps)
                        nc.vector.tensor_scalar_max(tA[:], tA[:], 1e-30)
                        tc1 = wk.tile([P, CF2], f32, tag="prep_c1", name="tc1")
                        nc.vector.reciprocal_approx_fast(tc1[:], tA[:])
                        nc.scalar.activation(t_c1b[a][:, csl], tc1[:],
                                             mybir.ActivationFunctionType.Copy)
                        # c0 = 1 - S*c1
                        tSc = wk.tile([P, CF2], f32, tag="prep_sc", name="tSc")
                        nc.gpsimd.tensor_tensor(out=tSc[:], in0=tS[:],
                                                in1=tc1[:], op=MULT)
                        nc.scalar.activation(t_c0[a][:, csl], tSc[:],
                                             mybir.ActivationFunctionType.Identity,
                                             bias=1.0, scale=-1.0)

            prep_axis("x")

            # ---- propagation steps ----
            # matmul order: center group (incl -S slot) first, then m1, p1 --
            # consecutive matmuls share the stationary shift matrix.
            MM_ORDER = [(3, 1), (4, 1),
                        (0, 0), (1, 0), (2, 0),
                        (5, 2), (6, 2), (7, 2)]
            step = 0
            for it in range(PROP_TIME):
                for a in AXES:
                    step += 1
                    if step == 2:
                        prep_axis("y")
                    elif step == 3:
                        prep_axis("z")
                    first = (step == 1)
                    dbu = ZC if a == "x" else 1
                    da_free = a == "z"

                    if not first:
                        # refresh y-overlap cols of r (SBUF->SBUF DMA;
                        # partition-offset copies are illegal on compute)
                        nc.sync.dma_start(out=t_r[S:P, 0, :],
                                          in_=t_r[0:P - S, YT, :])
                        nc.sync.dma_start(out=t_r[0:P - S, YC - 1, :],
                                          in_=t_r[S:P, 1, :])
                        # r3 slot1 = bf16(r); slot0/2 = shifted by -+dbu
                        rf = t_r[:].rearrange("p a b -> p (a b)")
                        nc.scalar.activation(
                            t_r3[:, 1, GUARD:GUARD + FD], rf,
                            mybir.ActivationFunctionType.Copy)
                        nc.scalar.activation(
                            t_r3[:, 0, GUARD:GUARD + FD],
                            t_r3[:, 1, GUARD - dbu:GUARD + FD - dbu],
                            mybir.ActivationFunctionType.Copy)
                        nc.scalar.activation(
                            t_r3[:, 2, GUARD:GUARD + FD],
                            t_r3[:, 1, GUARD + dbu:GUARD + FD + dbu],
                            mybir.ActivationFunctionType.Copy)

                    rfall = t_r[:].rearrange("p a b -> p (a b)")
                    nc.gpsimd.tensor_tensor(out=t_carry[:], in0=t_c0[a][:],
                                            in1=rfall, op=MULT)
                    for c in range(NCHUNK):
                        buf = (step * NCHUNK + c) % 2
                        dmae = nc.sync if c % 2 == 0 else nc.scalar
                        src_gs = gu["x"] if first else gs[a]
                        dmae.dma_start(out=t_g[buf][:], in_=src_gs[c])
                        tg_ = t_g[buf]
                        gsl = slice(0, CF)
                        if first:
                            stag_t = wk.tile([P, 3, 3, CF], bf16,
                                             tag="stagc", name="stag_t")
                            nc.sync.dma_start(out=stag_t[:], in_=r0stag[c])
                        # products per group (stacked over slots)
                        for gi, (s0, s1) in enumerate(GROUP_SLOTS):
                            nsl = s1 - s0
                            if first:
                                if nsl == 3:
                                    in1 = stag_t[:, gi, 0:3, :]
                                else:
                                    in1 = stag_t[:, gi, 0:3:2, :]
                            else:
                                base = GUARD + c * CF
                                if da_free:
                                    base += (gi - 1) * ZC
                                if nsl == 3:
                                    in1 = t_r3[:, 0:3, base:base + CF]
                                else:
                                    in1 = t_r3[:, 0:3:2, base:base + CF]
                            eng = nc.vector
                            eng.tensor_tensor(out=t_p[buf][:, s0:s1, :],
                                              in0=tg_[:, s0:s1, gsl],
                                              in1=in1, op=MULT)
                        # PE shift-matmul accumulate all 8 slots into PSUM
                        tps = pp.tile([P, CF], f32, tag="ps", name="tps")
                        for n0 in range(0, CF, 512):
                            n1 = min(CF, n0 + 512)
                            for mi, (s, gi) in enumerate(MM_ORDER):
                                smi = 1 if (first or da_free) else gi
                                nc.tensor.matmul(
                                    tps[:, n0:n1],
                                    t_shm[0:P, smi, 0:P],
                                    t_p[buf][:, s, n0:n1],
                                    start=(mi == 0), stop=(mi == 7))
                        # out chunk = c0*r + c1b*psum (writes r in place)
                        rfc = t_r[:].rearrange("p a b -> p (a b)")
                        tmul = fin.tile([P, CF], f32, tag="tmul", name="tmul")
                        nc.vector.tensor_tensor(
                            out=tmul[:],
                            in0=t_c1b[a][:, c * CF:(c + 1) * CF],
                            in1=tps[:], op=MULT)
                        nc.gpsimd.tensor_add(
                            out=rfc[:, c * CF:(c + 1) * CF],
                            in0=t_carry[:, c * CF:(c + 1) * CF],
                            in1=tmul[:])

            nc.sync.dma_start(out=rout[:], in_=t_r[:])

    nc.compile()
    return nc


def _prep_inputs(guidance, blur):
    """Host-side swizzle: build per-core input dicts."""
    guidance = np.asarray(guidance, dtype=np.float32)
    blur = np.asarray(blur, dtype=np.float32)[0]  # [X,Y,Z]
    x0s = [c * W for c in range(NCORES)]

    in_maps = [dict() for _ in range(NCORES)]

    # shift matrices: SM[q, g, m]: g=0: m=q+1 ; g=1: m=q ; g=2: m=q-1
    sm = np.zeros((128, 3, 128), dtype=BF)
    for q in range(P):
        if q + 1 < P:
            sm[q, 0, q + 1] = 1.0
        sm[q, 1, q] = 1.0
        if q - 1 >= 0:
            sm[q, 2, q - 1] = 1.0
    for c in range(NCORES):
        in_maps[c]["shm"] = sm

    # gate stacks, pre-shifted by full neighbor offset; the +-1 partition
    # (x) shift of the product routing is also baked per slot (slab start
    # x0 - da), except in the unbaked step-1 variant of axis x.
    for a in AXES:
        slots = _axis_slots(a)
        shifted = np.empty((8, X, Y, Z), dtype=np.float32)
        for si, (ch, dx, dy, dz) in enumerate(slots):
            shifted[si] = _shift_full(guidance[ch], dx, dy, dz)
        variants = [(f"gs_{a}", True), (f"gu_{a}", False)]
        for name, baked in variants:
            for c in range(NCORES):
                L = np.empty((P, 8, YC, ZC), dtype=np.float32)
                for si in range(8):
                    da = SLOT_DADB[si][0]
                    if a == "z" or not baked:
                        da = 0
                    L[:, si] = _slab_L(shifted[si], x0s[c] - da)
                Lh = L.reshape(P, 8, FD).reshape(P, 8, NCHUNK, CF)
                in_maps[c][name] = np.ascontiguousarray(
                    Lh.transpose(2, 0, 1, 3)).astype(BF)

    # r0 + staggered step-1 triples (axis x: da in x, db in y)
    for c in range(NCORES):
        in_maps[c]["r0"] = _slab_L(blur, x0s[c])
    stag = np.empty((3, 3, X, Y, Z), dtype=np.float32)
    for gi, da in enumerate((-1, 0, 1)):
        for j, db in enumerate((-1, 0, 1)):
            stag[gi, j] = _shift_full(blur, da, db, 0)
    for c in range(NCORES):
        stc = np.empty((P, 3, 3, FD), dtype=np.float32)
        for gi in range(3):
            for j in range(3):
                stc[:, gi, j] = _slab_L(stag[gi, j], x0s[c]).reshape(P, FD)
        stc = stc.reshape(P, 3, 3, NCHUNK, CF).transpose(3, 0, 1, 2, 4)
        in_maps[c]["r0stag"] = np.ascontiguousarray(stc).astype(BF)

    return in_maps


def _unswizzle(results):
    out = np.empty((1, X, Y, Z), dtype=np.float32)
    for c in range(NCORES):
        r = results[c]["rout"]  # [P, YC, ZC]
        x0 = c * W
        for yb in range(NYB):
            ys = yb * YT
            ye = min(Y, ys + YT)
            out[0, x0:x0 + W, ys:ye, :] = \
                r[yb * S + M: yb * S + M + W, 1:1 + (ye - ys), 1:Z + 1]
    return out


def kernel(guidance, blur):
    global _COMPILED, _LAST_RESULTS
    from concourse import bass_utils
    if _COMPILED is None:
        _COMPILED = _build_program()
    nc = _COMPILED
    in_maps = _prep_inputs(guidance, blur)
    res = bass_utils.run_bass_kernel_spmd(nc, in_maps,
                                          core_ids=list(range(NCORES)))
    _LAST_RESULTS = res
    return _unswizzle(res.results)



# revision 9
# speedup vs baseline: 1.2564x; 1.2564x over previous
"""Affinity-propagation (CSPN-3D) Trainium2 kernel, v2.

Problem: guidance [24,256,256,32] f32, blur [1,256,256,32] f32.
3 iterations of (x-plane, y-plane, z-plane) 8-neighbor gated propagation:

  out(q) = r(q) + c1(q) * [ sum_k G_k(q+d_k) * r(q+d_k) - S(q) * r(q) ]
  A(q) = sum_k |G_k(q+d_k)|, S(q) = sum_k G_k(q+d_k), c1 = 1/max(A,eps)

Sharding: 8 cores, X sharded 32 rows/core, ghost margin 5 rows each side;
step 1 consumes no margin (host supplies +-1-x-shifted blur slabs), the
remaining 5 x-crossing steps (axes x,y of iterations) consume 1 each.

Per-core layout: partitions p = yb*42 + xl (3 y-blocks x 42 x-rows = 126);
free dim f = ylocal*32 + z with ylocal in [0,88) (86-wide y-third + 1
overlap col each side) and z in [0,32) unpadded (z-boundary handled by
zero gates). FD = 2816, chunked 8 x 352 for pipelining.

Per step (all engines in parallel):
 - DMA: rm/rp = rc shifted +-1 partition (x-halo); y-ghost col refresh.
 - Act: rc = bf16(r) into a guarded window buffer.
 - DVE: one 9-slot product instruction per chunk:
       prods[p, da, j, f] = g[p, da, j, f] * rbuf_da[p, f + off(j)]
   where slot (da=0, j=center) holds nS = -S so the products include the
   -S*r center term.
 - PE: 9 identity-stationary matmuls accumulate the slots into PSUM f32.
 - Pool: t = c1 * psum ; r += t.

Gates are host-pre-shifted by their full 3D neighbor offset (pure layout;
no host arithmetic) and stay resident in SBUF in bf16 for axes x,y; axis z
gates stream per z-step. A/S/c1 (gate normalization) are computed on
device: |g| via a 4x-mode bitmask, slot sums via PE identity matmuls.
"""

import numpy as np
import ml_dtypes

BF = ml_dtypes.bfloat16

X = Y = 256
Z = 32
NCORES = 8
W = X // NCORES          # 32 interior x rows per core
M = 5                    # ghost margin rows
S = W + 2 * M            # 42 slab rows
NYB = 3                  # y thirds
YT = 86                  # y third width
YC = YT + 2              # y cols incl 1 overlap each side
ZC = Z                   # z cols, unpadded
FD = YC * ZC             # 2816
P = NYB * S              # 126 partitions
NCHUNK = 8
CF = FD // NCHUNK        # 352
GUARD = 34               # window guard (max offset ZC+1=33)
SLOTF = GUARD + FD + GUARD   # 2884
PROP_TIME = 3
EPS = 1e-30

# k -> (dH, dW) neighbor offsets, matching reference PADS
DLIST = [(1, 1), (1, 0), (1, -1), (0, 1), (0, -1), (-1, 1), (-1, 0), (-1, -1)]
# 3x3 slot enumeration (da, db), row-major; center (0,0) is the nS slot.
SLOT33 = [(-1, -1), (-1, 0), (-1, 1), (0, -1), (0, 0), (0, 1),
          (1, -1), (1, 0), (1, 1)]
HOST_SLOTS = [s for s in SLOT33 if s != (0, 0)]   # 8 real gate slots

AXES = ["x", "y", "z"]
# per-axis mapping of (da, db) onto (dx, dy, dz)
def _axis_d(axis, da, db):
    if axis == "x":
        return (da, db, 0)
    if axis == "y":
        return (da, 0, db)
    return (0, da, db)


# db free-dim stride per axis (axis z: da is also free, stride ZC)
DBU = {"x": ZC, "y": 1}


def _shift_full(f, dx, dy, dz):
    """Zero-padded shift: out[x,y,z] = f[x+dx, y+dy, z+dz]."""
    o = np.zeros_like(f)
    tx0, tx1 = max(0, -dx), min(X, X - dx)
    ty0, ty1 = max(0, -dy), min(Y, Y - dy)
    tz0, tz1 = max(0, -dz), min(Z, Z - dz)
    o[tx0:tx1, ty0:ty1, tz0:tz1] = f[tx0 + dx:tx1 + dx, ty0 + dy:ty1 + dy,
                                     tz0 + dz:tz1 + dz]
    return o


def _slab(f, x0):
    """Full field [X,Y,Z] -> core slab [P, FD] (f32).

    Rows x in [x0-M, x0-M+S); y blocks of 88 cols spanning global
    y in [86b-1, 86b+87) with zero pad at global y=-1/256."""
    pf = np.zeros((S, Y + 4, Z), dtype=np.float32)
    r0_, r1_ = x0 - M, x0 - M + S
    c0_, c1_ = max(0, r0_), min(X, r1_)
    pf[c0_ - r0_:c1_ - r0_, 1:Y + 1, :] = f[c0_:c1_]
    blocks = [pf[:, b * YT:b * YT + YC, :] for b in range(NYB)]
    return np.concatenate(blocks, axis=0).reshape(P, FD)


_COMPILED = None
_LAST_RESULTS = None


def _build_program():
    import concourse.bacc as bacc
    import concourse.mybir as mybir
    import concourse.tile as tile

    f32 = mybir.dt.float32
    bf16 = mybir.dt.bfloat16
    i16 = mybir.dt.int16
    MULT = mybir.AluOpType.mult
    ADD = mybir.AluOpType.add
    MAXOP = mybir.AluOpType.max
    AND = mybir.AluOpType.bitwise_and
    COPY = mybir.ActivationFunctionType.Copy

    nc = bacc.Bacc("TRN2", target_bir_lowering=False, debug=False,
                   num_devices=NCORES)

    for val in (-EPS, EPS):
        ct = nc.alloc_sbuf_tensor(f"const-f32-{val}", [128, 1], f32)
        nc.gpsimd.memset(ct.ap(), val)
        nc.const_aps.aps[(f32, val)] = ct.ap()

    # ---- DRAM I/O ----
    g_in = {a: nc.dram_tensor(f"g_{a}", [P, 8, FD], bf16,
                              kind="ExternalInput").ap() for a in ("x", "y")}
    gz_in = nc.dram_tensor("g_z", [NCHUNK, P, 8, CF], bf16,
                           kind="ExternalInput").ap()
    r0_in = nc.dram_tensor("r0", [P, FD], f32, kind="ExternalInput").ap()
    rm0_in = nc.dram_tensor("rm0", [P, SLOTF], bf16,
                            kind="ExternalInput").ap()
    rp0_in = nc.dram_tensor("rp0", [P, SLOTF], bf16,
                            kind="ExternalInput").ap()
    id_in = nc.dram_tensor("ident", [128, 128], bf16,
                           kind="ExternalInput").ap()
    rout = nc.dram_tensor("rout", [P, FD], f32, kind="ExternalOutput").ap()

    with tile.TileContext(nc) as tc:
        with tc.tile_pool(name="stat", bufs=1) as st, \
             tc.tile_pool(name="wk", bufs=2) as wk, \
             tc.tile_pool(name="fin", bufs=2) as fin, \
             tc.tile_pool(name="psum", bufs=2, space="PSUM") as pp, \
             tc.tile_pool(name="psprep", bufs=1, space="PSUM") as pq:

            # ---- static tiles ----
            t_g = {a: st.tile([P, 9, FD], bf16, tag=f"g{a}", name=f"t_g{a}")
                   for a in ("x", "y")}
            t_gz = [st.tile([P, 9, CF], bf16, tag=f"gz{i}", name=f"t_gz{i}")
                    for i in range(2)]
            t_nsz = st.tile([P, FD], bf16, tag="nsz", name="t_nsz")
            t_c1 = {a: st.tile([P, FD], bf16, tag=f"c1{a}", name=f"t_c1{a}")
                    for a in AXES}
            t_r = st.tile([P, FD], f32, tag="r", name="t_r")
            t_rs = st.tile([P, 3, SLOTF], bf16, tag="rs", name="t_rs")
            t_id = st.tile([128, 128], bf16, tag="id", name="t_id")
            t_p = [st.tile([P, 9, CF], bf16, tag=f"p{i}", name=f"t_p{i}")
                   for i in range(2)]

            APc = type(t_rs[:])
            rs_ap = t_rs[:]
            rs_pd = list(rs_ap.ap[0])
            rs_base = rs_ap.offset

            def win_xy(dbu, c):
                # [P, 3(buffer rm/rc/rp), 3(db window), CF]
                off = rs_base + GUARD + c * CF - dbu
                return APc(rs_ap.tensor, off,
                           [rs_pd, [SLOTF, 3], [dbu, 3], [1, CF]])

            def win_z(c):
                # all slots on rc (buf 1): [P, 3(dy win), 3(dz win), CF]
                off = rs_base + SLOTF + GUARD + c * CF - ZC - 1
                return APc(rs_ap.tensor, off,
                           [rs_pd, [ZC, 3], [1, 3], [1, CF]])

            # ---- init ----
            nc.sync.dma_start(out=t_id[:], in_=id_in[:])
            nc.gpsimd.memset(t_rs[:], 0.0)
            nc.sync.dma_start(out=t_r[:], in_=r0_in[:])
            nc.sync.dma_start(out=t_rs[:, 0, :], in_=rm0_in[:])
            nc.sync.dma_start(out=t_rs[:, 2, :], in_=rp0_in[:])

            def load_resident(a):
                for c in range(NCHUNK):
                    csl = slice(c * CF, (c + 1) * CF)
                    nc.scalar.dma_start(out=t_g[a][:, 0:4, csl],
                                        in_=g_in[a][:, 0:4, csl])
                    nc.scalar.dma_start(out=t_g[a][:, 5:9, csl],
                                        in_=g_in[a][:, 4:8, csl])

            def load_z_chunk(c, zbuf):
                nc.scalar.dma_start(out=t_gz[zbuf][:, 0:4, :],
                                    in_=gz_in[c][:, 0:4, :])
                nc.scalar.dma_start(out=t_gz[zbuf][:, 5:9, :],
                                    in_=gz_in[c][:, 4:8, :])

            def prep_chunk(a, c, gsrc, ns_dst):
                """Normalization for chunk c of axis a.
                gsrc: AP [P, 9, CF] (slots 0-3, 5-9 hold gates);
                ns_dst: AP [P, CF] bf16 to receive -S."""
                csl = slice(c * CF, (c + 1) * CF)
                psA = pq.tile([P, CF], f32, tag="psA", name="psA")
                psS = pq.tile([P, CF], f32, tag="psS", name="psS")
                slots = [0, 1, 2, 3, 5, 6, 7, 8]
                for h, sl in ((0, slice(0, 4)), (1, slice(5, 9))):
                    tabs = wk.tile([P, 4, CF], bf16, tag="tabs", name="tabs")
                    nc.vector.tensor_scalar(
                        tabs[:].bitcast(i16), gsrc[:, sl, :].bitcast(i16),
                        0x7FFF, None, AND)
                    for j in range(4):
                        nc.tensor.matmul(psA[:], t_id[0:P, 0:P],
                                         tabs[:, j, :],
                                         start=(h == 0 and j == 0),
                                         stop=(h == 1 and j == 3))
                for j, s in enumerate(slots):
                    nc.tensor.matmul(psS[:], t_id[0:P, 0:P], gsrc[:, s, :],
                                     start=(j == 0), stop=(j == 7))
                # c1 = 1/max(A, eps): Act does max via Relu(A-eps)+eps
                tA = wk.tile([P, CF], f32, tag="tA", name="tA")
                nc.scalar.activation(tA[:], psA[:],
                                     mybir.ActivationFunctionType.Relu,
                                     bias=-EPS, scale=1.0)
                nc.scalar.activation(tA[:], tA[:],
                                     mybir.ActivationFunctionType.Identity,
                                     bias=EPS, scale=1.0)
                tC = wk.tile([P, CF], f32, tag="tC", name="tC")
                nc.vector.reciprocal_approx_fast(tC[:], tA[:])
                nc.scalar.activation(t_c1[a][:, csl], tC[:], COPY)
                # nS = -S (bf16)
                nc.scalar.activation(ns_dst, psS[:], COPY, scale=-1.0)

            def emit_step(step, a):
                """One propagation step. step in 1..9."""
                zstep = a == "z"
                first = step == 1
                if not first:
                    # y-ghost col refresh (blocks share overlap cols)
                    rv = t_r[:].rearrange("p (y z) -> p y z", z=ZC)
                    nc.sync.dma_start(out=rv[S:P, 0, :],
                                      in_=rv[0:P - S, YT, :])
                    nc.sync.dma_start(out=rv[0:P - S, YC - 1, :],
                                      in_=rv[S:P, 1, :])
                # all rc/rm/rp copies BEFORE any product: a product's window
                # reads 1 col into the neighboring chunk, so the neighbor's
                # copy must be a true RAW dependency, not a later WAR write.
                for c in range(NCHUNK):
                    csl = slice(c * CF, (c + 1) * CF)
                    gsl = slice(GUARD + c * CF, GUARD + (c + 1) * CF)
                    # rc = bf16(r) (Act); step 1 reads r0 already in t_r
                    nc.scalar.activation(t_rs[:, 1, gsl], t_r[:, csl], COPY)
                    if not first:
                        # rm/rp partition shifts from rc
                        nc.sync.dma_start(out=t_rs[1:P, 0, gsl],
                                          in_=t_rs[0:P - 1, 1, gsl])
                        nc.sync.dma_start(out=t_rs[0:P - 1, 2, gsl],
                                          in_=t_rs[1:P, 1, gsl])
                for c in range(NCHUNK):
                    csl = slice(c * CF, (c + 1) * CF)
                    if zstep:
                        zbuf = c % 2
                        load_z_chunk(c, zbuf)
                        if step == 3:
                            prep_chunk("z", c, t_gz[zbuf][:],
                                       t_gz[zbuf][:, 4, :])
                            nc.vector.tensor_scalar(
                                t_nsz[:, csl], t_gz[zbuf][:, 4, :],
                                1.0, None, MULT)
                        else:
                            nc.vector.tensor_scalar(
                                t_gz[zbuf][:, 4, :], t_nsz[:, csl],
                                1.0, None, MULT)
                        gch = t_gz[zbuf][:]
                        in1 = win_z(c)
                    else:
                        gch = t_g[a][:, :, csl]
                        in1 = win_xy(DBU[a], c)
                    buf = (step * NCHUNK + c) % 2
                    nc.vector.tensor_tensor(
                        out=t_p[buf][:].rearrange("p (u v) f -> p u v f", u=3),
                        in0=gch.rearrange("p (u v) f -> p u v f", u=3),
                        in1=in1, op=MULT)
                    tps = pp.tile([P, CF], f32, tag="tps", name="tps")
                    for s2 in range(9):
                        nc.tensor.matmul(tps[:], t_id[0:P, 0:P],
                                         t_p[buf][:, s2, :],
                                         start=(s2 == 0), stop=(s2 == 8))
                    # combine: psb = bf16(psum) [Act]; t = c1*psb ; r += t
                    psb = fin.tile([P, CF], bf16, tag="psb", name="psb")
                    nc.scalar.activation(psb[:], tps[:], COPY)
                    tt = fin.tile([P, CF], bf16, tag="tt", name="tt")
                    nc.gpsimd.tensor_tensor(
                        out=tt[:], in0=psb[:], in1=t_c1[a][:, csl], op=MULT)
                    nc.gpsimd.tensor_tensor(
                        out=t_r[:, csl], in0=tt[:], in1=t_r[:, csl], op=ADD)

            # ---- schedule ----
            load_resident("x")
            for c in range(NCHUNK):
                csl = slice(c * CF, (c + 1) * CF)
                prep_chunk("x", c, t_g["x"][:, :, csl], t_g["x"][:, 4, csl])
            load_resident("y")
            emit_step(1, "x")
            for c in range(NCHUNK):
                csl = slice(c * CF, (c + 1) * CF)
                prep_chunk("y", c, t_g["y"][:, :, csl], t_g["y"][:, 4, csl])
            step = 1
            for it in range(PROP_TIME):
                for a in AXES:
                    if it == 0 and a == "x":
                        continue
                    step += 1
                    emit_step(step, a)

            nc.sync.dma_start(out=rout[:], in_=t_r[:])

    nc.compile()
    return nc


def _prep_inputs(guidance, blur):
    """Host-side swizzle: build per-core input dicts (layout only)."""
    guidance = np.asarray(guidance, dtype=np.float32)
    blur = np.asarray(blur, dtype=np.float32)[0]  # [X,Y,Z]
    x0s = [c * W for c in range(NCORES)]

    in_maps = [dict() for _ in range(NCORES)]

    ident = np.eye(128, dtype=BF)
    for c in range(NCORES):
        in_maps[c]["ident"] = ident

    for ai, a in enumerate(AXES):
        base = 8 * ai
        shifted = np.empty((8, X, Y, Z), dtype=np.float32)
        for si, (da, db) in enumerate(HOST_SLOTS):
            k = DLIST.index((da, db))
            dx, dy, dz = _axis_d(a, da, db)
            shifted[si] = _shift_full(guidance[base + k], dx, dy, dz)
        for c in range(NCORES):
            ga = np.empty((P, 8, FD), dtype=np.float32)
            for si in range(8):
                ga[:, si] = _slab(shifted[si], x0s[c])
            if a == "z":
                gaz = ga.reshape(P, 8, NCHUNK, CF).transpose(2, 0, 1, 3)
                in_maps[c]["g_z"] = np.ascontiguousarray(gaz).astype(BF)
            else:
                in_maps[c][f"g_{a}"] = ga.astype(BF)

    for c in range(NCORES):
        in_maps[c]["r0"] = _slab(blur, x0s[c]).astype(np.float32)
        for name, dx in (("rm0", -1), ("rp0", 1)):
            sl = np.zeros((P, SLOTF), dtype=BF)
            sl[:, GUARD:GUARD + FD] = _slab(blur, x0s[c] + dx).astype(BF)
            in_maps[c][name] = sl

    return in_maps


def _unswizzle(results):
    out = np.empty((1, X, Y, Z), dtype=np.float32)
    for c in range(NCORES):
        r = results[c]["rout"].reshape(P, YC, ZC)
        x0 = c * W
        for b in range(NYB):
            ys = b * YT
            ye = min(Y, ys + YT)
            out[0, x0:x0 + W, ys:ye, :] = \
                r[b * S + M: b * S + M + W, 1:1 + (ye - ys), :]
    return out


def kernel(guidance, blur):
    global _COMPILED, _LAST_RESULTS
    from concourse import bass_utils
    if _COMPILED is None:
        _COMPILED = _build_program()
    nc = _COMPILED
    in_maps = _prep_inputs(guidance, blur)
    res = bass_utils.run_bass_kernel_spmd(nc, in_maps,
                                          core_ids=list(range(NCORES)))
    _LAST_RESULTS = res
    return _unswizzle(res.results)


# revision 10
# speedup vs baseline: 1.2606x; 1.0033x over previous
"""Affinity-propagation (CSPN-3D) Trainium2 kernel, v3.

Problem: guidance [24,256,256,32] f32, blur [1,256,256,32] f32.
3 iterations of (x-plane, y-plane, z-plane) 8-neighbor gated propagation:

  out(q) = r(q) + c1(q) * [ sum_k G_k(q+d_k) * r(q+d_k) - S(q) * r(q) ]
  A(q) = sum_k |G_k(q+d_k)|, S(q) = sum_k G_k(q+d_k), c1 = 1/max(A,eps)

Sharding: 8 cores, X sharded 32 rows/core, ghost margin 5 rows each side;
step 1 consumes no margin (host supplies +-1-x-shifted blur slabs and
unbaked x gates), the remaining 5 x-crossing steps consume 1 each.

Per-core layout: partitions p = yb*42 + xl (3 y-blocks x 42 x-rows = 126);
free f = ylocal*32 + z, ylocal in [0,88) (86-wide y third + 1 overlap col
each side), z in [0,32) unpadded (z boundary handled by zero gates).
FD = 2816, chunked [512 x 5, 256].

Gates are host-pre-shifted by their full 3D offset and, for the x/y axes,
additionally "baked" by -da along partitions so products are computed
against the partition-local rc copy; a PE matmul with a +-1-shift
stationary routes each da group back while accumulating all 9 slots
(8 gates + a -S center slot) into PSUM. Per step:
 - Act: rc = bf16(r) into a guarded window buffer; psum -> bf16 drain.
 - DVE: one 9-slot product instruction per chunk (windowed 4-dim AP).
 - PE: 9 matmuls (3 stationaries) accumulate slots into PSUM f32.
 - Pool: t = c1*psum_bf16 ; r += t.
A/S/c1 are computed on device: |g| via 4x-mode bitmask (DVE), slot sums
via the same PE routing, 1/A via DVE reciprocal.
"""

import numpy as np
import ml_dtypes

BF = ml_dtypes.bfloat16

X = Y = 256
Z = 32
NCORES = 8
W = X // NCORES          # 32 interior x rows per core
M = 5                    # ghost margin rows
S = W + 2 * M            # 42 slab rows
NYB = 3                  # y thirds
YT = 86                  # y third width
YC = YT + 2              # y cols incl 1 overlap each side
ZC = Z                   # z cols, unpadded
FD = YC * ZC             # 2816
P = NYB * S              # 126 partitions
CHUNKS = [(0, 512), (512, 512), (1024, 512), (1536, 512),
          (2048, 512), (2560, 256)]
GUARD = 34               # window guard (max offset ZC+1=33)
SLOTF = GUARD + FD + GUARD   # 2884
PROP_TIME = 3
EPS = 1e-30

# k -> (dH, dW) neighbor offsets, matching reference PADS
DLIST = [(1, 1), (1, 0), (1, -1), (0, 1), (0, -1), (-1, 1), (-1, 0), (-1, -1)]
# 3x3 slot enumeration (da, db), row-major; center (0,0) is the nS slot.
SLOT33 = [(-1, -1), (-1, 0), (-1, 1), (0, -1), (0, 0), (0, 1),
          (1, -1), (1, 0), (1, 1)]
HOST_SLOTS = [s for s in SLOT33 if s != (0, 0)]   # 8 real gate slots
DEV_SLOT = [0, 1, 2, 3, 5, 6, 7, 8]               # device slot of HOST_SLOTS[i]
# matmul emission order: center group (identity) first, then da=-1, da=+1
MM_ORDER = [3, 4, 5, 0, 1, 2, 6, 7, 8]
SMI = {0: 0, 1: 0, 2: 0, 3: 1, 4: 1, 5: 1, 6: 2, 7: 2, 8: 2}

AXES = ["x", "y", "z"]


def _axis_d(axis, da, db):
    if axis == "x":
        return (da, db, 0)
    if axis == "y":
        return (da, 0, db)
    return (0, da, db)


# db free-dim stride per axis (axis z: da is also free with stride ZC)
DBU = {"x": ZC, "y": 1}


def _shift_full(f, dx, dy, dz):
    """Zero-padded shift: out[x,y,z] = f[x+dx, y+dy, z+dz]."""
    o = np.zeros_like(f)
    tx0, tx1 = max(0, -dx), min(X, X - dx)
    ty0, ty1 = max(0, -dy), min(Y, Y - dy)
    tz0, tz1 = max(0, -dz), min(Z, Z - dz)
    o[tx0:tx1, ty0:ty1, tz0:tz1] = f[tx0 + dx:tx1 + dx, ty0 + dy:ty1 + dy,
                                     tz0 + dz:tz1 + dz]
    return o


def _slab(f, x0):
    """Full field [X,Y,Z] -> core slab [P, FD] (f32)."""
    pf = np.zeros((S, Y + 4, Z), dtype=np.float32)
    r0_, r1_ = x0 - M, x0 - M + S
    c0_, c1_ = max(0, r0_), min(X, r1_)
    pf[c0_ - r0_:c1_ - r0_, 1:Y + 1, :] = f[c0_:c1_]
    blocks = [pf[:, b * YT:b * YT + YC, :] for b in range(NYB)]
    return np.concatenate(blocks, axis=0).reshape(P, FD)


_COMPILED = None
_LAST_RESULTS = None


def _build_program():
    import concourse.bacc as bacc
    import concourse.mybir as mybir
    import concourse.tile as tile

    f32 = mybir.dt.float32
    bf16 = mybir.dt.bfloat16
    i16 = mybir.dt.int16
    MULT = mybir.AluOpType.mult
    AND = mybir.AluOpType.bitwise_and
    COPY = mybir.ActivationFunctionType.Copy

    nc = bacc.Bacc("TRN2", target_bir_lowering=False, debug=False,
                   num_devices=NCORES)

    for val in (-EPS, EPS):
        ct = nc.alloc_sbuf_tensor(f"const-f32-{val}", [128, 1], f32)
        nc.gpsimd.memset(ct.ap(), val)
        nc.const_aps.aps[(f32, val)] = ct.ap()

    # ---- DRAM I/O ----
    g_in = {a: nc.dram_tensor(f"g_{a}", [P, 8, FD], bf16,
                              kind="ExternalInput").ap()
            for a in ("x", "y", "z")}
    gux_in = nc.dram_tensor("gux", [P, 6, FD], bf16,
                            kind="ExternalInput").ap()
    r0_in = nc.dram_tensor("r0", [P, FD], f32, kind="ExternalInput").ap()
    rm0_in = nc.dram_tensor("rm0", [P, SLOTF], bf16,
                            kind="ExternalInput").ap()
    rp0_in = nc.dram_tensor("rp0", [P, SLOTF], bf16,
                            kind="ExternalInput").ap()
    shm_in = nc.dram_tensor("shm", [128, 3, 128], bf16,
                            kind="ExternalInput").ap()
    rout = nc.dram_tensor("rout", [P, FD], f32, kind="ExternalOutput").ap()

    with tile.TileContext(nc) as tc:
        with tc.tile_pool(name="stat", bufs=1) as st, \
             tc.tile_pool(name="wk", bufs=2) as wk, \
             tc.tile_pool(name="fin", bufs=2) as fin, \
             tc.tile_pool(name="psum", bufs=2, space="PSUM") as pp, \
             tc.tile_pool(name="psprep", bufs=1, space="PSUM") as pq:

            # ---- static tiles ----
            t_g = {a: st.tile([P, 9, FD], bf16, tag=f"g{a}", name=f"t_g{a}")
                   for a in ("x", "y")}
            t_gz = [st.tile([P, 9, 512], bf16, tag=f"gz{i}", name=f"t_gz{i}")
                    for i in range(2)]
            t_nsz = st.tile([P, FD], bf16, tag="nsz", name="t_nsz")
            t_c1 = {a: st.tile([P, FD], bf16, tag=f"c1{a}", name=f"t_c1{a}")
                    for a in AXES}
            t_r = st.tile([P, FD], f32, tag="r", name="t_r")
            t_rs = st.tile([P, 3, SLOTF], bf16, tag="rs", name="t_rs")
            t_shm = st.tile([128, 3, 128], bf16, tag="shm", name="t_shm")
            t_p = [st.tile([P, 9, 512], bf16, tag=f"p{i}", name=f"t_p{i}")
                   for i in range(2)]

            APc = type(t_rs[:])
            rs_ap = t_rs[:]
            rs_pd = list(rs_ap.ap[0])
            rs_base = rs_ap.offset

            def win_rc(dbu, c0, cw):
                # all 9 slots on rc: [P, 3(da: routed, stride 0),
                #                     3(db win), cw]
                off = rs_base + SLOTF + GUARD + c0 - dbu
                return APc(rs_ap.tensor, off,
                           [rs_pd, [0, 3], [dbu, 3], [1, cw]])

            def win_z(c0, cw):
                # [P, 3(dy win), 3(dz win), cw] on rc
                off = rs_base + SLOTF + GUARD + c0 - ZC - 1
                return APc(rs_ap.tensor, off,
                           [rs_pd, [ZC, 3], [1, 3], [1, cw]])

            def win_s1(u, dbu, c0, cw):
                # step 1 group u: [P, 3(db win), cw] on host buffer u
                off = rs_base + u * SLOTF + GUARD + c0 - dbu
                return APc(rs_ap.tensor, off,
                           [rs_pd, [dbu, 3], [1, cw]])

            # ---- init ----
            nc.sync.dma_start(out=t_shm[:], in_=shm_in[:])
            nc.gpsimd.memset(t_rs[:], 0.0)
            nc.sync.dma_start(out=t_r[:], in_=r0_in[:])
            nc.sync.dma_start(out=t_rs[:, 0, :], in_=rm0_in[:])
            nc.sync.dma_start(out=t_rs[:, 2, :], in_=rp0_in[:])

            def load_resident(a):
                for c0, cw in CHUNKS:
                    csl = slice(c0, c0 + cw)
                    nc.scalar.dma_start(out=t_g[a][:, 0:4, csl],
                                        in_=g_in[a][:, 0:4, csl])
                    nc.scalar.dma_start(out=t_g[a][:, 5:9, csl],
                                        in_=g_in[a][:, 4:8, csl])

            def prep_chunk(a, ci, gsrc, ns_dst, smi_of):
                """Gate normalization for chunk ci of axis a.
                gsrc: AP [P, 9, cw] (slots 0-3, 5-9 hold gates);
                ns_dst: AP [P, cw] bf16 to receive -S."""
                c0, cw = CHUNKS[ci]
                csl = slice(c0, c0 + cw)
                psA = pq.tile([P, 512], f32, tag="psA", name="psA")
                psS = pq.tile([P, 512], f32, tag="psS", name="psS")
                for h, sl in ((0, slice(0, 4)), (1, slice(5, 9))):
                    tabs = wk.tile([P, 4, 512], bf16, tag="tabs", name="tabs")
                    nc.vector.tensor_scalar(
                        tabs[:, :, 0:cw].bitcast(i16),
                        gsrc[:, sl, :].bitcast(i16),
                        0x7FFF, None, AND)
                    for j in range(4):
                        s = (0, 1, 2, 3)[j] if h == 0 else (5, 6, 7, 8)[j]
                        nc.tensor.matmul(psA[:, 0:cw],
                                         t_shm[0:P, smi_of(s), 0:P],
                                         tabs[:, j, 0:cw],
                                         start=(h == 0 and j == 0),
                                         stop=(h == 1 and j == 3))
                for j, s in enumerate(DEV_SLOT):
                    nc.tensor.matmul(psS[:, 0:cw],
                                     t_shm[0:P, smi_of(s), 0:P],
                                     gsrc[:, s, :],
                                     start=(j == 0), stop=(j == 7))
                # c1 = 1/max(A, eps): Relu(A-eps)+eps is exact in f32
                tA = wk.tile([P, 512], f32, tag="tA", name="tA")
                nc.scalar.activation(tA[:, 0:cw], psA[:, 0:cw],
                                     mybir.ActivationFunctionType.Relu,
                                     bias=-EPS, scale=1.0)
                nc.scalar.activation(tA[:, 0:cw], tA[:, 0:cw],
                                     mybir.ActivationFunctionType.Identity,
                                     bias=EPS, scale=1.0)
                tC = wk.tile([P, 512], f32, tag="tC", name="tC")
                nc.vector.reciprocal_approx_fast(tC[:, 0:cw], tA[:, 0:cw])
                nc.scalar.activation(t_c1[a][:, csl], tC[:, 0:cw], COPY)
                # nS = -S (bf16)
                nc.scalar.activation(ns_dst, psS[:, 0:cw], COPY, scale=-1.0)

            def emit_step(step, a):
                """One propagation step. step in 1..9."""
                zstep = a == "z"
                first = step == 1
                if not first:
                    # y-ghost col refresh (blocks share overlap cols)
                    rv = t_r[:].rearrange("p (y z) -> p y z", z=ZC)
                    nc.sync.dma_start(out=rv[S:P, 0, :],
                                      in_=rv[0:P - S, YT, :])
                    nc.sync.dma_start(out=rv[0:P - S, YC - 1, :],
                                      in_=rv[S:P, 1, :])
                # all rc copies BEFORE any product: a product's window reads
                # into the neighboring chunk, so the neighbor's copy must be
                # a true RAW dependency, not a later WAR write.
                for c0, cw in CHUNKS:
                    nc.scalar.activation(
                        t_rs[:, 1, GUARD + c0:GUARD + c0 + cw],
                        t_r[:, c0:c0 + cw], COPY)
                for ci, (c0, cw) in enumerate(CHUNKS):
                    csl = slice(c0, c0 + cw)
                    if zstep:
                        zbuf = ci % 2
                        nc.scalar.dma_start(out=t_gz[zbuf][:, 0:4, 0:cw],
                                            in_=g_in["z"][:, 0:4, csl])
                        nc.scalar.dma_start(out=t_gz[zbuf][:, 5:9, 0:cw],
                                            in_=g_in["z"][:, 4:8, csl])
                        if step == 3:
                            prep_chunk("z", ci, t_gz[zbuf][:, :, 0:cw],
                                       t_gz[zbuf][:, 4, 0:cw], lambda s: 1)
                            nc.vector.tensor_scalar(
                                t_nsz[:, csl], t_gz[zbuf][:, 4, 0:cw],
                                1.0, None, MULT)
                        else:
                            nc.vector.tensor_scalar(
                                t_gz[zbuf][:, 4, 0:cw], t_nsz[:, csl],
                                1.0, None, MULT)
                    buf = (step + ci) % 2
                    if zstep:
                        nc.vector.tensor_tensor(
                            out=t_p[buf][:, :, 0:cw]
                            .rearrange("p (u v) f -> p u v f", u=3),
                            in0=t_gz[zbuf][:, :, 0:cw]
                            .rearrange("p (u v) f -> p u v f", u=3),
                            in1=win_z(c0, cw), op=MULT)
                    elif first:
                        # stream unbaked da=+-1 groups; center from resident
                        zbuf = ci % 2
                        nc.scalar.dma_start(out=t_gz[zbuf][:, 0:3, 0:cw],
                                            in_=gux_in[:, 0:3, csl])
                        nc.scalar.dma_start(out=t_gz[zbuf][:, 6:9, 0:cw],
                                            in_=gux_in[:, 3:6, csl])
                        for u, src in ((0, t_gz[zbuf]), (1, t_g[a]),
                                       (2, t_gz[zbuf])):
                            if u == 1:
                                in0 = src[:, 3:6, csl]
                            else:
                                in0 = src[:, 3 * u:3 * u + 3, 0:cw]
                            nc.vector.tensor_tensor(
                                out=t_p[buf][:, 3 * u:3 * u + 3, 0:cw],
                                in0=in0,
                                in1=win_s1(u, DBU[a], c0, cw), op=MULT)
                    else:
                        nc.vector.tensor_tensor(
                            out=t_p[buf][:, :, 0:cw]
                            .rearrange("p (u v) f -> p u v f", u=3),
                            in0=t_g[a][:, :, csl]
                            .rearrange("p (u v) f -> p u v f", u=3),
                            in1=win_rc(DBU[a], c0, cw), op=MULT)
                    tps = pp.tile([P, 512], f32, tag="tps", name="tps")
                    for mi, s in enumerate(MM_ORDER):
                        smi = 1 if (first or zstep) else SMI[s]
                        nc.tensor.matmul(tps[:, 0:cw], t_shm[0:P, smi, 0:P],
                                         t_p[buf][:, s, 0:cw],
                                         start=(mi == 0), stop=(mi == 8))
                    # combine: psb = bf16(psum) [Act]; t = c1*psb ; r += t
                    psb = fin.tile([P, 512], bf16, tag="psb", name="psb")
                    nc.scalar.activation(psb[:, 0:cw], tps[:, 0:cw], COPY)
                    tt = fin.tile([P, 512], bf16, tag="tt", name="tt")
                    nc.gpsimd.tensor_tensor(
                        out=tt[:, 0:cw], in0=psb[:, 0:cw],
                        in1=t_c1[a][:, csl], op=MULT)
                    nc.gpsimd.tensor_tensor(
                        out=t_r[:, csl], in0=tt[:, 0:cw],
                        in1=t_r[:, csl], op=MULT if False else
                        mybir.AluOpType.add)

            # ---- schedule ----
            load_resident("x")
            for ci in range(len(CHUNKS)):
                c0, cw = CHUNKS[ci]
                csl = slice(c0, c0 + cw)
                prep_chunk("x", ci, t_g["x"][:, :, csl],
                           t_g["x"][:, 4, csl], lambda s: SMI[s])
            load_resident("y")
            emit_step(1, "x")
            for ci in range(len(CHUNKS)):
                c0, cw = CHUNKS[ci]
                csl = slice(c0, c0 + cw)
                prep_chunk("y", ci, t_g["y"][:, :, csl],
                           t_g["y"][:, 4, csl], lambda s: SMI[s])
            step = 1
            for it in range(PROP_TIME):
                for a in AXES:
                    if it == 0 and a == "x":
                        continue
                    step += 1
                    emit_step(step, a)

            nc.sync.dma_start(out=rout[:], in_=t_r[:])

    nc.compile()
    return nc


def _prep_inputs(guidance, blur):
    """Host-side swizzle: build per-core input dicts (layout only)."""
    guidance = np.asarray(guidance, dtype=np.float32)
    blur = np.asarray(blur, dtype=np.float32)[0]  # [X,Y,Z]
    x0s = [c * W for c in range(NCORES)]

    in_maps = [dict() for _ in range(NCORES)]

    # shift matrices sm[p, g, q]: route product at partition p=q+da -> q.
    # g=0 (da=-1): q=p+1 ; g=1: q=p ; g=2 (da=+1): q=p-1
    sm = np.zeros((128, 3, 128), dtype=BF)
    for q in range(P):
        if q - 1 >= 0:
            sm[q - 1, 0, q] = 1.0
        sm[q, 1, q] = 1.0
        if q + 1 < P:
            sm[q + 1, 2, q] = 1.0
    for c in range(NCORES):
        in_maps[c]["shm"] = sm

    for ai, a in enumerate(AXES):
        base = 8 * ai
        shifted = np.empty((8, X, Y, Z), dtype=np.float32)
        for si, (da, db) in enumerate(HOST_SLOTS):
            k = DLIST.index((da, db))
            dx, dy, dz = _axis_d(a, da, db)
            shifted[si] = _shift_full(guidance[base + k], dx, dy, dz)
        for c in range(NCORES):
            ga = np.empty((P, 8, FD), dtype=np.float32)
            for si, (da, db) in enumerate(HOST_SLOTS):
                bake = da if a in ("x", "y") else 0
                ga[:, si] = _slab(shifted[si], x0s[c] - bake)
            in_maps[c][f"g_{a}"] = ga.astype(BF)
            if a == "x":
                gu = np.empty((P, 6, FD), dtype=np.float32)
                for j, si in enumerate((0, 1, 2, 5, 6, 7)):
                    gu[:, j] = _slab(shifted[si], x0s[c])
                in_maps[c]["gux"] = gu.astype(BF)

    for c in range(NCORES):
        in_maps[c]["r0"] = _slab(blur, x0s[c]).astype(np.float32)
        for name, dx in (("rm0", -1), ("rp0", 1)):
            sl = np.zeros((P, SLOTF), dtype=BF)
            sl[:, GUARD:GUARD + FD] = _slab(blur, x0s[c] + dx).astype(BF)
            in_maps[c][name] = sl

    return in_maps


def _unswizzle(results):
    out = np.empty((1, X, Y, Z), dtype=np.float32)
    for c in range(NCORES):
        r = results[c]["rout"].reshape(P, YC, ZC)
        x0 = c * W
        for b in range(NYB):
            ys = b * YT
            ye = min(Y, ys + YT)
            out[0, x0:x0 + W, ys:ye, :] = \
                r[b * S + M: b * S + M + W, 1:1 + (ye - ys), :]
    return out


def kernel(guidance, blur):
    global _COMPILED, _LAST_RESULTS
    from concourse import bass_utils
    if _COMPILED is None:
        _COMPILED = _build_program()
    nc = _COMPILED
    in_maps = _prep_inputs(guidance, blur)
    res = bass_utils.run_bass_kernel_spmd(nc, in_maps,
                                          core_ids=list(range(NCORES)))
    _LAST_RESULTS = res
    return _unswizzle(res.results)


# revision 17
# speedup vs baseline: 1.3601x; 1.0790x over previous
"""Affinity-propagation (CSPN-3D) Trainium2 kernel, v3.

Problem: guidance [24,256,256,32] f32, blur [1,256,256,32] f32.
3 iterations of (x-plane, y-plane, z-plane) 8-neighbor gated propagation:

  out(q) = r(q) + c1(q) * [ sum_k G_k(q+d_k) * r(q+d_k) - S(q) * r(q) ]
  A(q) = sum_k |G_k(q+d_k)|, S(q) = sum_k G_k(q+d_k), c1 = 1/max(A,eps)

Sharding: 8 cores, X sharded 32 rows/core, ghost margin 5 rows each side;
step 1 consumes no margin (host supplies +-1-x-shifted blur slabs and
unbaked x gates), the remaining 5 x-crossing steps consume 1 each.

Per-core layout: partitions p = yb*42 + xl (3 y-blocks x 42 x-rows = 126);
free f = ylocal*32 + z, ylocal in [0,88) (86-wide y third + 1 overlap col
each side), z in [0,32) unpadded (z boundary handled by zero gates).
FD = 2816, chunked [512 x 5, 256].

Gates are host-pre-shifted by their full 3D offset and, for the x/y axes,
additionally "baked" by -da along partitions so products are computed
against the partition-local rc copy; a PE matmul with a +-1-shift
stationary routes each da group back while accumulating all 9 slots
(8 gates + a -S center slot) into PSUM. Per step:
 - Act: rc = bf16(r) into a guarded window buffer; psum -> bf16 drain.
 - DVE: one 9-slot product instruction per chunk (windowed 4-dim AP).
 - PE: 9 matmuls (3 stationaries) accumulate slots into PSUM f32.
 - Pool: t = c1*psum_bf16 ; r += t.
A/S/c1 are computed on device: |g| via 4x-mode bitmask (DVE), slot sums
via the same PE routing, 1/A via DVE reciprocal.
"""

import numpy as np
import ml_dtypes

BF = ml_dtypes.bfloat16

X = Y = 256
Z = 32
NCORES = 8
W = X // NCORES          # 32 interior x rows per core
M = 5                    # ghost margin rows
S = W + 2 * M            # 42 slab rows
NYB = 3                  # y thirds
YT = 86                  # y third width
YC = YT + 2              # y cols incl 1 overlap each side
ZC = Z                   # z cols, unpadded
FD = YC * ZC             # 2816
P = NYB * S              # 126 partitions
CHUNKS = [(0, 512), (512, 512), (1024, 512), (1536, 512),
          (2048, 512), (2560, 256)]
# processing order: ghost-source chunks (0: col 1, 5: col 86) first so the
# next step's ghost refresh isn't gated on this step's tail
CORDER = [0, 5, 1, 2, 3, 4]
GUARD = 34               # window guard (max offset ZC+1=33)
SLOTF = GUARD + FD + GUARD   # 2884
PROP_TIME = 3
EPS = 1e-30

# k -> (dH, dW) neighbor offsets, matching reference PADS
DLIST = [(1, 1), (1, 0), (1, -1), (0, 1), (0, -1), (-1, 1), (-1, 0), (-1, -1)]
# 3x3 slot enumeration (da, db), row-major; center (0,0) is the nS slot.
SLOT33 = [(-1, -1), (-1, 0), (-1, 1), (0, -1), (0, 0), (0, 1),
          (1, -1), (1, 0), (1, 1)]
HOST_SLOTS = [s for s in SLOT33 if s != (0, 0)]   # 8 real gate slots
DEV_SLOT = [0, 1, 2, 3, 5, 6, 7, 8]               # device slot of HOST_SLOTS[i]
# matmul emission order: center group (identity) first, then da=-1, da=+1
MM_ORDER = [3, 4, 5, 0, 1, 2, 6, 7, 8]
SMI = {0: 0, 1: 0, 2: 0, 3: 1, 4: 1, 5: 1, 6: 2, 7: 2, 8: 2}

AXES = ["x", "y", "z"]


def _axis_d(axis, da, db):
    if axis == "x":
        return (da, db, 0)
    if axis == "y":
        return (da, 0, db)
    return (0, da, db)


# db free-dim stride per axis (axis z: da is also free with stride ZC)
DBU = {"x": ZC, "y": 1}


def _shift_full(f, dx, dy, dz):
    """Zero-padded shift: out[x,y,z] = f[x+dx, y+dy, z+dz]."""
    o = np.zeros_like(f)
    tx0, tx1 = max(0, -dx), min(X, X - dx)
    ty0, ty1 = max(0, -dy), min(Y, Y - dy)
    tz0, tz1 = max(0, -dz), min(Z, Z - dz)
    o[tx0:tx1, ty0:ty1, tz0:tz1] = f[tx0 + dx:tx1 + dx, ty0 + dy:ty1 + dy,
                                     tz0 + dz:tz1 + dz]
    return o


def _slab(f, x0):
    """Full field [X,Y,Z] -> core slab [P, FD] (f32)."""
    pf = np.zeros((S, Y + 4, Z), dtype=np.float32)
    r0_, r1_ = x0 - M, x0 - M + S
    c0_, c1_ = max(0, r0_), min(X, r1_)
    pf[c0_ - r0_:c1_ - r0_, 1:Y + 1, :] = f[c0_:c1_]
    blocks = [pf[:, b * YT:b * YT + YC, :] for b in range(NYB)]
    return np.concatenate(blocks, axis=0).reshape(P, FD)


_COMPILED = None
_LAST_RESULTS = None


def _build_program():
    import concourse.bacc as bacc
    import concourse.mybir as mybir
    import concourse.tile as tile

    f32 = mybir.dt.float32
    bf16 = mybir.dt.bfloat16
    i16 = mybir.dt.int16
    MULT = mybir.AluOpType.mult
    AND = mybir.AluOpType.bitwise_and
    COPY = mybir.ActivationFunctionType.Copy

    nc = bacc.Bacc("TRN2", target_bir_lowering=False, debug=False,
                   num_devices=NCORES)

    for val in (-EPS, EPS):
        ct = nc.alloc_sbuf_tensor(f"const-f32-{val}", [128, 1], f32)
        nc.gpsimd.memset(ct.ap(), val)
        nc.const_aps.aps[(f32, val)] = ct.ap()

    # ---- DRAM I/O ----
    g_in = {a: nc.dram_tensor(f"g_{a}", [P, 8, FD], bf16,
                              kind="ExternalInput").ap()
            for a in ("x", "y", "z")}
    gux_in = nc.dram_tensor("gux", [P, 6, FD], bf16,
                            kind="ExternalInput").ap()
    r0_in = nc.dram_tensor("r0", [P, FD], f32, kind="ExternalInput").ap()
    rm0_in = nc.dram_tensor("rm0", [P, SLOTF], bf16,
                            kind="ExternalInput").ap()
    rp0_in = nc.dram_tensor("rp0", [P, SLOTF], bf16,
                            kind="ExternalInput").ap()
    shm_in = nc.dram_tensor("shm", [128, 3, 128], bf16,
                            kind="ExternalInput").ap()
    rout = nc.dram_tensor("rout", [P, FD], f32, kind="ExternalOutput").ap()

    with tile.TileContext(nc) as tc:
        with tc.tile_pool(name="stat", bufs=1) as st, \
             tc.tile_pool(name="wk", bufs=2) as wk, \
             tc.tile_pool(name="fin", bufs=3) as fin, \
             tc.tile_pool(name="psum", bufs=3, space="PSUM") as pp, \
             tc.tile_pool(name="psprep", bufs=1, space="PSUM") as pq:

            # ---- static tiles ----
            t_g = {a: st.tile([P, 9, FD], bf16, tag=f"g{a}", name=f"t_g{a}")
                   for a in ("x", "y")}
            t_gz = [st.tile([P, 9, 512], bf16, tag=f"gz{i}", name=f"t_gz{i}")
                    for i in range(2)]
            t_nsz = st.tile([P, FD], bf16, tag="nsz", name="t_nsz")
            t_c1 = {a: st.tile([P, FD], bf16, tag=f"c1{a}", name=f"t_c1{a}")
                    for a in AXES}
            t_r = st.tile([P, FD], f32, tag="r", name="t_r")
            t_rs = st.tile([P, 3, SLOTF], bf16, tag="rs", name="t_rs")
            t_shm = st.tile([128, 3, 128], bf16, tag="shm", name="t_shm")
            t_p = [st.tile([P, 9, 512], bf16, tag=f"p{i}", name=f"t_p{i}")
                   for i in range(2)]

            APc = type(t_rs[:])
            rs_ap = t_rs[:]
            rs_pd = list(rs_ap.ap[0])
            rs_base = rs_ap.offset

            def win_rc(dbu, c0, cw):
                # all 9 slots on rc: [P, 3(da: routed, stride 0),
                #                     3(db win), cw]
                off = rs_base + SLOTF + GUARD + c0 - dbu
                return APc(rs_ap.tensor, off,
                           [rs_pd, [0, 3], [dbu, 3], [1, cw]])

            def win_z(c0, cw):
                # [P, 3(dy win), 3(dz win), cw] on rc
                off = rs_base + SLOTF + GUARD + c0 - ZC - 1
                return APc(rs_ap.tensor, off,
                           [rs_pd, [ZC, 3], [1, 3], [1, cw]])

            def win_s1(u, dbu, c0, cw):
                # step 1 group u: [P, 3(db win), cw] on host buffer u
                off = rs_base + u * SLOTF + GUARD + c0 - dbu
                return APc(rs_ap.tensor, off,
                           [rs_pd, [dbu, 3], [1, cw]])

            # ---- init ----
            nc.sync.dma_start(out=t_shm[:], in_=shm_in[:])
            nc.gpsimd.memset(t_rs[:], 0.0)
            nc.sync.dma_start(out=t_r[:], in_=r0_in[:])
            nc.sync.dma_start(out=t_rs[:, 0, :], in_=rm0_in[:])
            nc.sync.dma_start(out=t_rs[:, 2, :], in_=rp0_in[:])

            def load_resident(a):
                for ci in CORDER:
                    c0, cw = CHUNKS[ci]
                    csl = slice(c0, c0 + cw)
                    nc.scalar.dma_start(out=t_g[a][:, 0:4, csl],
                                        in_=g_in[a][:, 0:4, csl])
                    nc.scalar.dma_start(out=t_g[a][:, 5:9, csl],
                                        in_=g_in[a][:, 4:8, csl])

            def prep_chunk(a, ci, gsrc, ns_dst, smi_of):
                """Gate normalization for chunk ci of axis a.
                gsrc: AP [P, 9, cw] (slots 0-3, 5-9 hold gates);
                ns_dst: AP [P, cw] bf16 to receive -S."""
                c0, cw = CHUNKS[ci]
                csl = slice(c0, c0 + cw)
                psA = pq.tile([P, 512], f32, tag="psA", name="psA")
                psS = pq.tile([P, 512], f32, tag="psS", name="psS")
                for h, sl in ((0, slice(0, 4)), (1, slice(5, 9))):
                    tabs = wk.tile([P, 4, 512], bf16, tag="tabs", name="tabs")
                    nc.vector.tensor_scalar(
                        tabs[:, :, 0:cw].bitcast(i16),
                        gsrc[:, sl, :].bitcast(i16),
                        0x7FFF, None, AND)
                    for j in range(4):
                        s = (0, 1, 2, 3)[j] if h == 0 else (5, 6, 7, 8)[j]
                        nc.tensor.matmul(psA[:, 0:cw],
                                         t_shm[0:P, smi_of(s), 0:P],
                                         tabs[:, j, 0:cw],
                                         start=(h == 0 and j == 0),
                                         stop=(h == 1 and j == 3))
                for j, s in enumerate(DEV_SLOT):
                    nc.tensor.matmul(psS[:, 0:cw],
                                     t_shm[0:P, smi_of(s), 0:P],
                                     gsrc[:, s, :],
                                     start=(j == 0), stop=(j == 7))
                # c1 = 1/max(A, eps): Relu(A-eps)+eps is exact in f32
                tA = wk.tile([P, 512], f32, tag="tA", name="tA")
                nc.scalar.activation(tA[:, 0:cw], psA[:, 0:cw],
                                     mybir.ActivationFunctionType.Relu,
                                     bias=-EPS, scale=1.0)
                nc.scalar.activation(tA[:, 0:cw], tA[:, 0:cw],
                                     mybir.ActivationFunctionType.Identity,
                                     bias=EPS, scale=1.0)
                tC = wk.tile([P, 512], f32, tag="tC", name="tC")
                nc.vector.reciprocal_approx_fast(tC[:, 0:cw], tA[:, 0:cw])
                nc.scalar.activation(t_c1[a][:, csl], tC[:, 0:cw], COPY)
                # nS = -S (bf16)
                nc.scalar.activation(ns_dst, psS[:, 0:cw], COPY, scale=-1.0)

            def emit_step(step, a, pre_chunk=None, post_chunk=None):
                """One propagation step. step in 1..9."""
                zstep = a == "z"
                first = step == 1
                if not first:
                    # y-ghost col refresh (blocks share overlap cols)
                    rv = t_r[:].rearrange("p (y z) -> p y z", z=ZC)
                    nc.sync.dma_start(out=rv[S:P, 0, :],
                                      in_=rv[0:P - S, YT, :])
                    nc.sync.dma_start(out=rv[0:P - S, YC - 1, :],
                                      in_=rv[S:P, 1, :])
                # all rc copies BEFORE any product: a product's window reads
                # into the neighboring chunk, so the neighbor's copy must be
                # a true RAW dependency, not a later WAR write. Ghost cols
                # (0 and 87) are copied separately so the bulk copies don't
                # wait on the ghost refresh DMAs.
                for c0, cw in CHUNKS:
                    b0, b1 = max(c0, ZC), min(c0 + cw, FD - ZC)
                    nc.scalar.activation(
                        t_rs[:, 1, GUARD + b0:GUARD + b1],
                        t_r[:, b0:b1], COPY)
                nc.scalar.activation(t_rs[:, 1, GUARD:GUARD + ZC],
                                     t_r[:, 0:ZC], COPY)
                nc.scalar.activation(t_rs[:, 1, GUARD + FD - ZC:GUARD + FD],
                                     t_r[:, FD - ZC:FD], COPY)
                for pos, ci in enumerate(CORDER):
                    c0, cw = CHUNKS[ci]
                    csl = slice(c0, c0 + cw)
                    if pre_chunk is not None:
                        pre_chunk(ci)
                    if zstep:
                        zbuf = pos % 2
                        nc.scalar.dma_start(out=t_gz[zbuf][:, 0:4, 0:cw],
                                            in_=g_in["z"][:, 0:4, csl])
                        nc.scalar.dma_start(out=t_gz[zbuf][:, 5:9, 0:cw],
                                            in_=g_in["z"][:, 4:8, csl])
                        if step == 3:
                            prep_chunk("z", ci, t_gz[zbuf][:, :, 0:cw],
                                       t_gz[zbuf][:, 4, 0:cw], lambda s: 1)
                            nc.vector.tensor_scalar(
                                t_nsz[:, csl], t_gz[zbuf][:, 4, 0:cw],
                                1.0, None, MULT)
                        else:
                            nc.vector.tensor_scalar(
                                t_gz[zbuf][:, 4, 0:cw], t_nsz[:, csl],
                                1.0, None, MULT)
                    buf = (step + pos) % 2
                    if zstep:
                        nc.vector.tensor_tensor(
                            out=t_p[buf][:, :, 0:cw]
                            .rearrange("p (u v) f -> p u v f", u=3),
                            in0=t_gz[zbuf][:, :, 0:cw]
                            .rearrange("p (u v) f -> p u v f", u=3),
                            in1=win_z(c0, cw), op=MULT)
                    elif first:
                        # stream unbaked da=+-1 groups; center from resident
                        zbuf = pos % 2
                        nc.sync.dma_start(out=t_gz[zbuf][:, 0:3, 0:cw],
                                          in_=gux_in[:, 0:3, csl])
                        nc.sync.dma_start(out=t_gz[zbuf][:, 6:9, 0:cw],
                                          in_=gux_in[:, 3:6, csl])
                        for u, src in ((0, t_gz[zbuf]), (1, t_g[a]),
                                       (2, t_gz[zbuf])):
                            if u == 1:
                                in0 = src[:, 3:6, csl]
                            else:
                                in0 = src[:, 3 * u:3 * u + 3, 0:cw]
                            nc.vector.tensor_tensor(
                                out=t_p[buf][:, 3 * u:3 * u + 3, 0:cw],
                                in0=in0,
                                in1=win_s1(u, DBU[a], c0, cw), op=MULT)
                    else:
                        nc.vector.tensor_tensor(
                            out=t_p[buf][:, :, 0:cw]
                            .rearrange("p (u v) f -> p u v f", u=3),
                            in0=t_g[a][:, :, csl]
                            .rearrange("p (u v) f -> p u v f", u=3),
                            in1=win_rc(DBU[a], c0, cw), op=MULT)
                    tps = pp.tile([P, 512], f32, tag="tps", name="tps")
                    for mi, s in enumerate(MM_ORDER):
                        smi = 1 if (first or zstep) else SMI[s]
                        nc.tensor.matmul(tps[:, 0:cw], t_shm[0:P, smi, 0:P],
                                         t_p[buf][:, s, 0:cw],
                                         start=(mi == 0), stop=(mi == 8))
                    # combine: psb = bf16(psum) [Act]; t = c1*psb ; r += t
                    psb = fin.tile([P, 512], bf16, tag="psb", name="psb")
                    nc.scalar.activation(psb[:, 0:cw], tps[:, 0:cw], COPY)
                    tt = fin.tile([P, 512], bf16, tag="tt", name="tt")
                    nc.gpsimd.tensor_tensor(
                        out=tt[:, 0:cw], in0=psb[:, 0:cw],
                        in1=t_c1[a][:, csl], op=MULT)
                    nc.gpsimd.tensor_tensor(
                        out=t_r[:, csl], in0=tt[:, 0:cw],
                        in1=t_r[:, csl], op=mybir.AluOpType.add)
                    if post_chunk is not None:
                        post_chunk(ci)

            # ---- schedule ----
            def prep_of(a):
                def f(ci):
                    c0, cw = CHUNKS[ci]
                    csl = slice(c0, c0 + cw)
                    prep_chunk(a, ci, t_g[a][:, :, csl],
                               t_g[a][:, 4, csl], lambda s: SMI[s])
                return f

            load_resident("x")
            load_resident("y")
            # prep-x feeds step 1's combine; prep-y rides along to keep the
            # PE busy during step 1's product stalls.
            emit_step(1, "x", pre_chunk=prep_of("x"), post_chunk=prep_of("y"))
            step = 1
            for it in range(PROP_TIME):
                for a in AXES:
                    if it == 0 and a == "x":
                        continue
                    step += 1
                    emit_step(step, a)

            nc.sync.dma_start(out=rout[:], in_=t_r[:])

    nc.compile()
    return nc


def _prep_inputs(guidance, blur):
    """Host-side swizzle: build per-core input dicts (layout only)."""
    guidance = np.asarray(guidance, dtype=np.float32)
    blur = np.asarray(blur, dtype=np.float32)[0]  # [X,Y,Z]
    x0s = [c * W for c in range(NCORES)]

    in_maps = [dict() for _ in range(NCORES)]

    # shift matrices sm[p, g, q]: route product at partition p=q+da -> q.
    # g=0 (da=-1): q=p+1 ; g=1: q=p ; g=2 (da=+1): q=p-1
    sm = np.zeros((128, 3, 128), dtype=BF)
    for q in range(P):
        if q - 1 >= 0:
            sm[q - 1, 0, q] = 1.0
        sm[q, 1, q] = 1.0
        if q + 1 < P:
            sm[q + 1, 2, q] = 1.0
    for c in range(NCORES):
        in_maps[c]["shm"] = sm

    for ai, a in enumerate(AXES):
        base = 8 * ai
        shifted = np.empty((8, X, Y, Z), dtype=np.float32)
        for si, (da, db) in enumerate(HOST_SLOTS):
            k = DLIST.index((da, db))
            dx, dy, dz = _axis_d(a, da, db)
            shifted[si] = _shift_full(guidance[base + k], dx, dy, dz)
        for c in range(NCORES):
            ga = np.empty((P, 8, FD), dtype=np.float32)
            for si, (da, db) in enumerate(HOST_SLOTS):
                bake = da if a in ("x", "y") else 0
                ga[:, si] = _slab(shifted[si], x0s[c] - bake)
            in_maps[c][f"g_{a}"] = ga.astype(BF)
            if a == "x":
                gu = np.empty((P, 6, FD), dtype=np.float32)
                for j, si in enumerate((0, 1, 2, 5, 6, 7)):
                    gu[:, j] = _slab(shifted[si], x0s[c])
                in_maps[c]["gux"] = gu.astype(BF)

    for c in range(NCORES):
        in_maps[c]["r0"] = _slab(blur, x0s[c]).astype(np.float32)
        for name, dx in (("rm0", -1), ("rp0", 1)):
            sl = np.zeros((P, SLOTF), dtype=BF)
            sl[:, GUARD:GUARD + FD] = _slab(blur, x0s[c] + dx).astype(BF)
            in_maps[c][name] = sl

    return in_maps


def _unswizzle(results):
    out = np.empty((1, X, Y, Z), dtype=np.float32)
    for c in range(NCORES):
        r = results[c]["rout"].reshape(P, YC, ZC)
        x0 = c * W
        for b in range(NYB):
            ys = b * YT
            ye = min(Y, ys + YT)
            out[0, x0:x0 + W, ys:ye, :] = \
                r[b * S + M: b * S + M + W, 1:1 + (ye - ys), :]
    return out


def kernel(guidance, blur):
    global _COMPILED, _LAST_RESULTS
    from concourse import bass_utils
    if _COMPILED is None:
        _COMPILED = _build_program()
    nc = _COMPILED
    in_maps = _prep_inputs(guidance, blur)
    res = bass_utils.run_bass_kernel_spmd(nc, in_maps,
                                          core_ids=list(range(NCORES)))
    _LAST_RESULTS = res
    return _unswizzle(res.results)


# revision 26
# speedup vs baseline: 1.4579x; 1.0719x over previous
"""Affinity-propagation (CSPN-3D) Trainium2 kernel, v3.

Problem: guidance [24,256,256,32] f32, blur [1,256,256,32] f32.
3 iterations of (x-plane, y-plane, z-plane) 8-neighbor gated propagation:

  out(q) = r(q) + c1(q) * [ sum_k G_k(q+d_k) * r(q+d_k) - S(q) * r(q) ]
  A(q) = sum_k |G_k(q+d_k)|, S(q) = sum_k G_k(q+d_k), c1 = 1/max(A,eps)

Sharding: 8 cores, X sharded 32 rows/core, ghost margin 5 rows each side;
step 1 consumes no margin (host supplies +-1-x-shifted blur slabs and
unbaked x gates), the remaining 5 x-crossing steps consume 1 each.

Per-core layout: partitions p = yb*42 + xl (3 y-blocks x 42 x-rows = 126);
free f = ylocal*32 + z, ylocal in [0,88) (86-wide y third + 1 overlap col
each side), z in [0,32) unpadded (z boundary handled by zero gates).
FD = 2816, chunked [512 x 5, 256].

Gates are host-pre-shifted by their full 3D offset and, for the x/y axes,
additionally "baked" by -da along partitions so products are computed
against the partition-local rc copy; a PE matmul with a +-1-shift
stationary routes each da group back while accumulating all 9 slots
(8 gates + a -S center slot) into PSUM. Per step:
 - Act: rc = bf16(r) into a guarded window buffer; psum -> bf16 drain.
 - DVE: one 9-slot product instruction per chunk (windowed 4-dim AP).
 - PE: 9 matmuls (3 stationaries) accumulate slots into PSUM f32.
 - Pool: t = c1*psum_bf16 ; r += t.
A/S/c1 are computed on device: |g| via 4x-mode bitmask (DVE), slot sums
via the same PE routing, 1/A via DVE reciprocal.
"""

import numpy as np
import ml_dtypes

BF = ml_dtypes.bfloat16

X = Y = 256
Z = 32
NCORES = 8
W = X // NCORES          # 32 interior x rows per core
M = 5                    # ghost margin rows
S = W + 2 * M            # 42 slab rows
NYB = 3                  # y thirds
YT = 86                  # y third width
YC = YT + 2              # y cols incl 1 overlap each side
ZC = Z                   # z cols, unpadded
FD = YC * ZC             # 2816
P = NYB * S              # 126 partitions
CHUNKS = [(0, 512), (512, 512), (1024, 512), (1536, 512),
          (2048, 512), (2560, 256)]
# processing order: ghost-source chunks (0: col 1, 5: col 86) first so the
# next step's ghost refresh isn't gated on this step's tail
CORDER = [0, 5, 1, 2, 3, 4]
GUARD = 34               # window guard (max offset ZC+1=33)
SLOTF = GUARD + FD + GUARD   # 2884
PROP_TIME = 3
EPS = 1e-30

# k -> (dH, dW) neighbor offsets, matching reference PADS
DLIST = [(1, 1), (1, 0), (1, -1), (0, 1), (0, -1), (-1, 1), (-1, 0), (-1, -1)]
# 3x3 slot enumeration (da, db), row-major; center (0,0) is the nS slot.
SLOT33 = [(-1, -1), (-1, 0), (-1, 1), (0, -1), (0, 0), (0, 1),
          (1, -1), (1, 0), (1, 1)]
HOST_SLOTS = [s for s in SLOT33 if s != (0, 0)]   # 8 real gate slots
DEV_SLOT = [0, 1, 2, 3, 5, 6, 7, 8]               # device slot of HOST_SLOTS[i]
# matmul emission order: center group (identity) first, then da=-1, da=+1
MM_ORDER = [3, 4, 5, 0, 1, 2, 6, 7, 8]
SMI = {0: 0, 1: 0, 2: 0, 3: 1, 4: 1, 5: 1, 6: 2, 7: 2, 8: 2}

AXES = ["x", "y", "z"]


def _axis_d(axis, da, db):
    if axis == "x":
        return (da, db, 0)
    if axis == "y":
        return (da, 0, db)
    return (0, da, db)


# db free-dim stride per axis (axis z: da is also free with stride ZC)
DBU = {"x": ZC, "y": 1}


def _shift_full(f, dx, dy, dz):
    """Zero-padded shift: out[x,y,z] = f[x+dx, y+dy, z+dz]."""
    o = np.zeros_like(f)
    tx0, tx1 = max(0, -dx), min(X, X - dx)
    ty0, ty1 = max(0, -dy), min(Y, Y - dy)
    tz0, tz1 = max(0, -dz), min(Z, Z - dz)
    o[tx0:tx1, ty0:ty1, tz0:tz1] = f[tx0 + dx:tx1 + dx, ty0 + dy:ty1 + dy,
                                     tz0 + dz:tz1 + dz]
    return o


def _slab(f, x0):
    """Full field [X,Y,Z] -> core slab [P, FD] (f32)."""
    pf = np.zeros((S, Y + 4, Z), dtype=np.float32)
    r0_, r1_ = x0 - M, x0 - M + S
    c0_, c1_ = max(0, r0_), min(X, r1_)
    pf[c0_ - r0_:c1_ - r0_, 1:Y + 1, :] = f[c0_:c1_]
    blocks = [pf[:, b * YT:b * YT + YC, :] for b in range(NYB)]
    return np.concatenate(blocks, axis=0).reshape(P, FD)


_COMPILED = None
_LAST_RESULTS = None


def _build_program():
    import concourse.bacc as bacc
    import concourse.mybir as mybir
    import concourse.tile as tile

    f32 = mybir.dt.float32
    bf16 = mybir.dt.bfloat16
    i16 = mybir.dt.int16
    MULT = mybir.AluOpType.mult
    AND = mybir.AluOpType.bitwise_and
    COPY = mybir.ActivationFunctionType.Copy

    nc = bacc.Bacc("TRN2", target_bir_lowering=False, debug=False,
                   num_devices=NCORES)

    for val in (-EPS, EPS):
        ct = nc.alloc_sbuf_tensor(f"const-f32-{val}", [128, 1], f32)
        nc.gpsimd.memset(ct.ap(), val)
        nc.const_aps.aps[(f32, val)] = ct.ap()

    # ---- DRAM I/O ----
    g_in = {a: nc.dram_tensor(f"g_{a}", [P, 8, FD], bf16,
                              kind="ExternalInput").ap()
            for a in ("x", "y", "z")}
    gux_in = nc.dram_tensor("gux", [P, 6, FD], bf16,
                            kind="ExternalInput").ap()
    r0_in = nc.dram_tensor("r0", [P, FD], f32, kind="ExternalInput").ap()
    rm0_in = nc.dram_tensor("rm0", [P, SLOTF], bf16,
                            kind="ExternalInput").ap()
    rp0_in = nc.dram_tensor("rp0", [P, SLOTF], bf16,
                            kind="ExternalInput").ap()
    shm_in = nc.dram_tensor("shm", [128, 3, 128], bf16,
                            kind="ExternalInput").ap()
    rout = nc.dram_tensor("rout", [P, FD], f32, kind="ExternalOutput").ap()

    with tile.TileContext(nc) as tc:
        with tc.tile_pool(name="stat", bufs=1) as st, \
             tc.tile_pool(name="wk", bufs=2) as wk, \
             tc.tile_pool(name="fin", bufs=3) as fin, \
             tc.tile_pool(name="psum", bufs=3, space="PSUM") as pp, \
             tc.tile_pool(name="psprep", bufs=1, space="PSUM") as pq:

            # ---- static tiles ----
            t_g = {a: st.tile([P, 9, FD], bf16, tag=f"g{a}", name=f"t_g{a}")
                   for a in ("x", "y")}
            t_gz = [st.tile([P, 9, 512], bf16, tag=f"gz{i}", name=f"t_gz{i}")
                    for i in range(2)]
            t_nsz = st.tile([P, FD], bf16, tag="nsz", name="t_nsz")
            t_c1 = {a: st.tile([P, FD], bf16, tag=f"c1{a}", name=f"t_c1{a}")
                    for a in AXES}
            t_r = st.tile([P, FD], f32, tag="r", name="t_r")
            t_rs = st.tile([P, 3, SLOTF], bf16, tag="rs", name="t_rs")
            t_shm = st.tile([128, 3, 128], bf16, tag="shm", name="t_shm")
            t_p = [st.tile([P, 9, 512], bf16, tag=f"p{i}", name=f"t_p{i}")
                   for i in range(2)]
            t_tt = st.tile([P, 6, 512], bf16, tag="tt6", name="t_tt")

            APc = type(t_rs[:])
            rs_ap = t_rs[:]
            rs_pd = list(rs_ap.ap[0])
            rs_base = rs_ap.offset

            def win_rc(dbu, c0, cw):
                # all 9 slots on rc: [P, 3(da: routed, stride 0),
                #                     3(db win), cw]
                off = rs_base + SLOTF + GUARD + c0 - dbu
                return APc(rs_ap.tensor, off,
                           [rs_pd, [0, 3], [dbu, 3], [1, cw]])

            def win_z(c0, cw):
                # [P, 3(dy win), 3(dz win), cw] on rc
                off = rs_base + SLOTF + GUARD + c0 - ZC - 1
                return APc(rs_ap.tensor, off,
                           [rs_pd, [ZC, 3], [1, 3], [1, cw]])

            def win_s1(u, dbu, c0, cw):
                # step 1 group u: [P, 3(db win), cw] on host buffer u
                off = rs_base + u * SLOTF + GUARD + c0 - dbu
                return APc(rs_ap.tensor, off,
                           [rs_pd, [dbu, 3], [1, cw]])

            # ---- init ----
            nc.sync.dma_start(out=t_shm[:], in_=shm_in[:])
            nc.gpsimd.memset(t_rs[:], 0.0)
            nc.sync.dma_start(out=t_r[:], in_=r0_in[:])
            nc.sync.dma_start(out=t_rs[:, 0, :], in_=rm0_in[:])
            nc.sync.dma_start(out=t_rs[:, 2, :], in_=rp0_in[:])

            def load_resident(a):
                for ci in CORDER:
                    c0, cw = CHUNKS[ci]
                    csl = slice(c0, c0 + cw)
                    nc.scalar.dma_start(out=t_g[a][:, 0:4, csl],
                                        in_=g_in[a][:, 0:4, csl])
                    nc.scalar.dma_start(out=t_g[a][:, 5:9, csl],
                                        in_=g_in[a][:, 4:8, csl])

            def prep_chunk(a, ci, gsrc, ns_dst, smi_of):
                """Gate normalization for chunk ci of axis a.
                gsrc: AP [P, 9, cw] (slots 0-3, 5-9 hold gates);
                ns_dst: AP [P, cw] bf16 to receive -S."""
                c0, cw = CHUNKS[ci]
                csl = slice(c0, c0 + cw)
                psA = pq.tile([P, 512], f32, tag="psA", name="psA")
                psS = pq.tile([P, 512], f32, tag="psS", name="psS")
                for h, sl in ((0, slice(0, 4)), (1, slice(5, 9))):
                    tabs = wk.tile([P, 4, 512], bf16, tag="tabs", name="tabs")
                    nc.vector.tensor_scalar(
                        tabs[:, :, 0:cw].bitcast(i16),
                        gsrc[:, sl, :].bitcast(i16),
                        0x7FFF, None, AND)
                    for j in range(4):
                        s = (0, 1, 2, 3)[j] if h == 0 else (5, 6, 7, 8)[j]
                        nc.tensor.matmul(psA[:, 0:cw],
                                         t_shm[0:P, smi_of(s), 0:P],
                                         tabs[:, j, 0:cw],
                                         start=(h == 0 and j == 0),
                                         stop=(h == 1 and j == 3))
                for j, s in enumerate(DEV_SLOT):
                    nc.tensor.matmul(psS[:, 0:cw],
                                     t_shm[0:P, smi_of(s), 0:P],
                                     gsrc[:, s, :],
                                     start=(j == 0), stop=(j == 7))
                # c1 = 1/max(A, eps): Relu(A-eps)+eps is exact in f32
                tA = wk.tile([P, 512], f32, tag="tA", name="tA")
                nc.scalar.activation(tA[:, 0:cw], psA[:, 0:cw],
                                     mybir.ActivationFunctionType.Relu,
                                     bias=-EPS, scale=1.0)
                nc.scalar.activation(tA[:, 0:cw], tA[:, 0:cw],
                                     mybir.ActivationFunctionType.Identity,
                                     bias=EPS, scale=1.0)
                nc.vector.reciprocal_approx_fast(tA[:, 0:cw], tA[:, 0:cw])
                nc.scalar.activation(t_c1[a][:, csl], tA[:, 0:cw], COPY)
                # nS = -S (bf16)
                nc.scalar.activation(ns_dst, psS[:, 0:cw], COPY, scale=-1.0)

            def emit_step(step, a, pre_chunk=None, post_chunk=None):
                """One propagation step. step in 1..9."""
                zstep = a == "z"
                first = step == 1
                if not first:
                    # y-ghost col refresh in rc space (blocks share overlap)
                    gi = GUARD
                    nc.sync.dma_start(
                        out=t_rs[S:P, 1, gi:gi + ZC],
                        in_=t_rs[0:P - S, 1, gi + YT * ZC:gi + YT * ZC + ZC])
                    nc.sync.dma_start(
                        out=t_rs[0:P - S, 1, gi + FD - ZC:gi + FD],
                        in_=t_rs[S:P, 1, gi + ZC:gi + 2 * ZC])
                for pos, ci in enumerate(CORDER):
                    c0, cw = CHUNKS[ci]
                    csl = slice(c0, c0 + cw)
                    if pre_chunk is not None:
                        pre_chunk(ci)
                    if zstep:
                        zbuf = pos % 2
                        if pos >= 2:   # pos 0,1 were prefetched
                            nc.scalar.dma_start(out=t_gz[zbuf][:, 0:4, 0:cw],
                                                in_=g_in["z"][:, 0:4, csl])
                            nc.scalar.dma_start(out=t_gz[zbuf][:, 5:9, 0:cw],
                                                in_=g_in["z"][:, 4:8, csl])
                        if step == 3:
                            prep_chunk("z", ci, t_gz[zbuf][:, :, 0:cw],
                                       t_gz[zbuf][:, 4, 0:cw], lambda s: 1)
                            nc.vector.tensor_scalar(
                                t_nsz[:, csl], t_gz[zbuf][:, 4, 0:cw],
                                1.0, None, MULT)
                        else:
                            nc.vector.tensor_scalar(
                                t_gz[zbuf][:, 4, 0:cw], t_nsz[:, csl],
                                1.0, None, MULT)
                    buf = (step + pos) % 2
                    if zstep:
                        nc.vector.tensor_tensor(
                            out=t_p[buf][:, :, 0:cw]
                            .rearrange("p (u v) f -> p u v f", u=3),
                            in0=t_gz[zbuf][:, :, 0:cw]
                            .rearrange("p (u v) f -> p u v f", u=3),
                            in1=win_z(c0, cw), op=MULT)
                    elif first:
                        # stream unbaked da=+-1 groups; center from resident
                        zbuf = pos % 2
                        nc.sync.dma_start(out=t_gz[zbuf][:, 0:3, 0:cw],
                                          in_=gux_in[:, 0:3, csl])
                        nc.sync.dma_start(out=t_gz[zbuf][:, 6:9, 0:cw],
                                          in_=gux_in[:, 3:6, csl])
                        for u, src in ((0, t_gz[zbuf]), (1, t_g[a]),
                                       (2, t_gz[zbuf])):
                            if u == 1:
                                in0 = src[:, 3:6, csl]
                            else:
                                in0 = src[:, 3 * u:3 * u + 3, 0:cw]
                            nc.vector.tensor_tensor(
                                out=t_p[buf][:, 3 * u:3 * u + 3, 0:cw],
                                in0=in0,
                                in1=win_s1(u, DBU[a], c0, cw), op=MULT)
                    else:
                        nc.vector.tensor_tensor(
                            out=t_p[buf][:, :, 0:cw]
                            .rearrange("p (u v) f -> p u v f", u=3),
                            in0=t_g[a][:, :, csl]
                            .rearrange("p (u v) f -> p u v f", u=3),
                            in1=win_rc(DBU[a], c0, cw), op=MULT)
                    tps = pp.tile([P, 512], f32, tag="tps", name="tps")
                    for mi, s in enumerate(MM_ORDER):
                        smi = 1 if (first or zstep) else SMI[s]
                        nc.tensor.matmul(tps[:, 0:cw], t_shm[0:P, smi, 0:P],
                                         t_p[buf][:, s, 0:cw],
                                         start=(mi == 0), stop=(mi == 8))
                    # combine: psb = bf16(psum) [Act]; t = c1*psb [Pool];
                    # rc += t [DVE, the propagating bf16 state];
                    # r_f32 += t [Pool, off the critical path, output only]
                    psb = fin.tile([P, 512], bf16, tag="psb", name="psb")
                    nc.scalar.activation(psb[:, 0:cw], tps[:, 0:cw], COPY)
                    tt = t_tt[:, ci, :]
                    nc.gpsimd.tensor_tensor(
                        out=tt[:, 0:cw], in0=psb[:, 0:cw],
                        in1=t_c1[a][:, csl], op=MULT)
                    nc.gpsimd.tensor_tensor(
                        out=t_r[:, csl], in0=tt[:, 0:cw],
                        in1=t_r[:, csl], op=mybir.AluOpType.add)
                    if post_chunk is not None:
                        post_chunk(ci)
                # rc updates AFTER all products: a product's window reads the
                # neighboring chunk's edge cols, which must still hold this
                # step's input state when it runs (DVE executes in order).
                for ci in CORDER:
                    c0, cw = CHUNKS[ci]
                    rcc = t_rs[:, 1, GUARD + c0:GUARD + c0 + cw]
                    nc.vector.tensor_tensor(
                        out=rcc, in0=t_tt[:, ci, 0:cw], in1=rcc,
                        op=mybir.AluOpType.add)

            # ---- schedule ----
            def prep_of(a):
                def f(ci):
                    c0, cw = CHUNKS[ci]
                    csl = slice(c0, c0 + cw)
                    prep_chunk(a, ci, t_g[a][:, :, csl],
                               t_g[a][:, 4, csl], lambda s: SMI[s])
                return f

            def z_prefetch():
                for pos in range(2):
                    ci = CORDER[pos]
                    c0, cw = CHUNKS[ci]
                    csl = slice(c0, c0 + cw)
                    nc.scalar.dma_start(out=t_gz[pos][:, 0:4, 0:cw],
                                        in_=g_in["z"][:, 0:4, csl])
                    nc.scalar.dma_start(out=t_gz[pos][:, 5:9, 0:cw],
                                        in_=g_in["z"][:, 4:8, csl])

            def step1_post(ci):
                c0, cw = CHUNKS[ci]
                csl = slice(c0, c0 + cw)
                nc.scalar.dma_start(out=t_g["y"][:, 0:4, csl],
                                    in_=g_in["y"][:, 0:4, csl])
                nc.scalar.dma_start(out=t_g["y"][:, 5:9, csl],
                                    in_=g_in["y"][:, 4:8, csl])
                prep_of("y")(ci)

            load_resident("x")
            # prefix: rc = bf16(r0)
            for c0, cw in CHUNKS:
                nc.scalar.activation(
                    t_rs[:, 1, GUARD + c0:GUARD + c0 + cw],
                    t_r[:, c0:c0 + cw], COPY)
            # prep-x feeds step 1's combine; prep-y rides along to keep the
            # PE busy during step 1's product stalls.
            emit_step(1, "x", pre_chunk=prep_of("x"), post_chunk=step1_post)
            step = 1
            for it in range(PROP_TIME):
                for a in AXES:
                    if it == 0 and a == "x":
                        continue
                    step += 1
                    if a == "z":
                        z_prefetch()
                    emit_step(step, a)

            nc.sync.dma_start(out=rout[:], in_=t_r[:])

    nc.compile()
    return nc


def _prep_inputs(guidance, blur):
    """Host-side swizzle: build per-core input dicts (layout only)."""
    guidance = np.asarray(guidance, dtype=np.float32)
    blur = np.asarray(blur, dtype=np.float32)[0]  # [X,Y,Z]
    x0s = [c * W for c in range(NCORES)]

    in_maps = [dict() for _ in range(NCORES)]

    # shift matrices sm[p, g, q]: route product at partition p=q+da -> q.
    # g=0 (da=-1): q=p+1 ; g=1: q=p ; g=2 (da=+1): q=p-1
    sm = np.zeros((128, 3, 128), dtype=BF)
    for q in range(P):
        if q - 1 >= 0:
            sm[q - 1, 0, q] = 1.0
        sm[q, 1, q] = 1.0
        if q + 1 < P:
            sm[q + 1, 2, q] = 1.0
    for c in range(NCORES):
        in_maps[c]["shm"] = sm

    for ai, a in enumerate(AXES):
        base = 8 * ai
        shifted = np.empty((8, X, Y, Z), dtype=np.float32)
        for si, (da, db) in enumerate(HOST_SLOTS):
            k = DLIST.index((da, db))
            dx, dy, dz = _axis_d(a, da, db)
            shifted[si] = _shift_full(guidance[base + k], dx, dy, dz)
        for c in range(NCORES):
            ga = np.empty((P, 8, FD), dtype=np.float32)
            for si, (da, db) in enumerate(HOST_SLOTS):
                bake = da if a in ("x", "y") else 0
                ga[:, si] = _slab(shifted[si], x0s[c] - bake)
            in_maps[c][f"g_{a}"] = ga.astype(BF)
            if a == "x":
                gu = np.empty((P, 6, FD), dtype=np.float32)
                for j, si in enumerate((0, 1, 2, 5, 6, 7)):
                    gu[:, j] = _slab(shifted[si], x0s[c])
                in_maps[c]["gux"] = gu.astype(BF)

    for c in range(NCORES):
        in_maps[c]["r0"] = _slab(blur, x0s[c]).astype(np.float32)
        for name, dx in (("rm0", -1), ("rp0", 1)):
            sl = np.zeros((P, SLOTF), dtype=BF)
            sl[:, GUARD:GUARD + FD] = _slab(blur, x0s[c] + dx).astype(BF)
            in_maps[c][name] = sl

    return in_maps


def _unswizzle(results):
    out = np.empty((1, X, Y, Z), dtype=np.float32)
    for c in range(NCORES):
        r = results[c]["rout"].reshape(P, YC, ZC)
        x0 = c * W
        for b in range(NYB):
            ys = b * YT
            ye = min(Y, ys + YT)
            out[0, x0:x0 + W, ys:ye, :] = \
                r[b * S + M: b * S + M + W, 1:1 + (ye - ys), :]
    return out


def kernel(guidance, blur):
    global _COMPILED, _LAST_RESULTS
    from concourse import bass_utils
    if _COMPILED is None:
        _COMPILED = _build_program()
    nc = _COMPILED
    in_maps = _prep_inputs(guidance, blur)
    res = bass_utils.run_bass_kernel_spmd(nc, in_maps,
                                          core_ids=list(range(NCORES)))
    _LAST_RESULTS = res
    return _unswizzle(res.results)
